# revision 1
# baseline (speedup 1.0000x reference)
"""Trainium2 Bass kernel for nn_ActionSmoothingLoss.

Math (per row y of previous_actions, x = segmented log_softmax(current_action)):
    e = exp(y)                       (no max-subtraction: |y| <= ~5.5, safe in f32)
    Z_j = sum_{i in seg j} e_i
    S_j = sum_{i in seg j} e_i * (y_i - x_i)
    loss = (1/W) * sum_rows sum_j inv_n_j * [ S_j / Z_j - log Z_j ]

Sharding: data-parallel over W across 8 cores; x replicated; partial sums
(per-partition accumulators) gathered and combined on host in float64.

Device pipeline per tile [128 partitions, rt rows x 68] with the tile
schedule rt = [64, 64, 128, 128, 128] (small first for a short DMA ramp;
large after — per-tile fixed costs are ~3.5us of instr overhead + sems):
    ScalarE: e = exp(y) -> bf16; grouped per-inv_n Ln(Z) accums into accB.
    DVE:     d = y - x          (x via 0-stride broadcast AP, bf16 out)
             cums = MUL_CUMSUM(e, d)   <- custom DVE op: prefix-sum of the
                    product e*d in one 1x pass (fuses multiply + S-reduce)
             smp  = cums sampled at the 6 segment-end columns of each row
                    (3 strided copies: end cols {2,5},{9,34,59},{67})
             S    = adjacent-difference of smp  (exact per-(row,seg) sums:
                    the cumsum is continuous across rows, so diffs of
                    consecutive segment-end samples telescope correctly)
             Z    = per-segment tensor_reduces over e (adjacent equal-n
                    segments share one reduce via 4D APs: 4 instrs)
             rzi  = reciprocal_approx_fast(Z) * inv_n_broadcast
             stt  = S * rzi with accum into accA (1 instr)
Measured: 147.3us HW (baseline was 214us harness / 180.9us traced); DVE is
~90% busy at its floor of 3 full-tile 1x passes (sub, cumsum, Z-reduce).
Measured dead ends: GpSimd offload (shares DVE's SBUF port: 223us),
ScalarE sampling (cross-engine SBUF contention: 178us), PE/PSUM subtract
(per-matmul LDWEIGHTS + PSUM ping-pong lockstep: 152us), bf16 reduces
(tensor_reduce has only a 1x uop).
"""

import sys

sys.path.insert(0, "/opt/trn_rl_repo")

import numpy as np

NVEC = (3, 3, 4, 25, 25, 8)
OFFS = (0, 3, 6, 10, 35, 60)
ENDS = (2, 5, 9, 34, 59, 67)  # inclusive end column of each segment
A = 68
P = 128
N_CORES = 8
W_FULL = 524288
W_CORE = W_FULL // N_CORES  # 65536
R = 64                      # rows per partition per tile
F = R * A                   # 4352 free elems per tile
T = W_CORE // (P * R)       # 8 tiles per core

_PROGRAM_CACHE = {}
_MUL_CUMSUM = None


def _register_mul_cumsum():
    """Register the MUL_CUMSUM_ANT custom DVE op (out = cumsum(in0*in1) along
    the free dim, fp32 state). Uses the documented extension point
    (dve_ops.OPS); the uop table ships inside the NEFF so no firmware change
    is involved. Idempotent."""
    global _MUL_CUMSUM
    if _MUL_CUMSUM is not None:
        return _MUL_CUMSUM
    import concourse.dve_ops as dve_ops_mod
    from concourse.dve_spec import Spec, Src0, Src1, AluOp, scan, lower
    from concourse.dve_uop import DveOpSpec

    NAME = "MUL_CUMSUM_ANT"
    for op in dve_ops_mod.OPS:
        if op.name == NAME:
            _MUL_CUMSUM = op
            return op

    def _ref(in0, in1, s0, s1, imm2):
        p = in0.shape[0]
        prod = (np.asarray(in0, np.float32).reshape(p, -1)
                * np.asarray(in1, np.float32).reshape(p, -1)).astype(np.float32)
        return np.cumsum(prod, axis=-1, dtype=np.float32)

    spec = Spec(body=scan(AluOp.ADD, Src0 * Src1), reference=_ref)
    row = dve_ops_mod._CUSTOM_DVE_ROW_BASE + len(dve_ops_mod.OPS)
    assert row < 0x20
    shas = {}
    for ver in ("v3",):
        s = DveOpSpec(name=NAME, opcode=row, uops=lower(spec, ver=ver), rd1_en=True)
        shas[ver] = s.sha(ver)
    op = dve_ops_mod.DveOp(NAME, spec, subdim=False, uops_sha=shas)
    dve_ops_mod.OPS.append(op)
    dve_ops_mod._SUB_OPCODE_FOR_NAME[NAME] = row
    dve_ops_mod.CUSTOM_DVE_SPECS[NAME] = spec
    _MUL_CUMSUM = op
    return op


def build_program(w_core=W_CORE, r=R):
    import concourse.bass as bass
    import concourse.bacc as bacc
    import concourse.mybir as mybir
    from concourse import tile

    mul_cumsum = _register_mul_cumsum()

    f32 = mybir.dt.float32
    bf16 = mybir.dt.bfloat16
    # Tile schedule: small tiles first (short DMA ramp), large tiles after
    # (per-tile fixed costs — instruction overheads + semaphores — are ~3.8us,
    # so fewer/larger tiles win once the pipeline is primed).
    rows_pp = w_core // P  # rows per partition
    if rows_pp >= 384 and (rows_pp - 128) % 128 == 0:
        RS = [64, 64] + [128] * ((rows_pp - 128) // 128)
    else:
        RS = [64] * (rows_pp // 64)
    assert sum(RS) == rows_pp
    r_max = max(RS)
    Fmax = r_max * A
    Tt = len(RS)

    Exp = mybir.ActivationFunctionType.Exp
    Ln = mybir.ActivationFunctionType.Ln
    sub_op = mybir.AluOpType.subtract
    mult_op = mybir.AluOpType.mult
    add_op = mybir.AluOpType.add
    AX = mybir.AxisListType.X

    nc = bacc.Bacc(None, target_bir_lowering=False)
    pa = nc.dram_tensor("pa", [w_core, A], f32, kind="ExternalInput")
    # xb carries x broadcast (cols 0..67) plus the 6 inv_n values (68..73).
    xb = nc.dram_tensor("xb", [P, A + 6], f32, kind="ExternalInput")
    acc_a = nc.dram_tensor("acc_a", [P, Tt], f32, kind="ExternalOutput")
    acc_b = nc.dram_tensor("acc_b", [P, Tt * 4], f32, kind="ExternalOutput")

    # [P, rows_pp*A] view: partition p holds its rows contiguously; tile t
    # takes the next rt rows of every partition.
    pav = pa.rearrange("(p q) a -> p (q a)", p=P)

    with tile.TileContext(nc) as tc:
        with tc.tile_pool(name="io", bufs=2) as io, \
             tc.tile_pool(name="wk", bufs=2) as wk, \
             tc.tile_pool(name="cm", bufs=1) as cm, \
             tc.tile_pool(name="sm", bufs=1) as sm, \
             tc.tile_pool(name="zp", bufs=2) as zp, \
             tc.tile_pool(name="ps", bufs=1) as ps:
            xbt = ps.tile([P, A + 6], f32)
            nc.sync.dma_start(xbt[:], xb[:], single_packet=True)
            accA = ps.tile([P, Tt], f32)
            accB = ps.tile([P, Tt * 4], f32)
            row0 = 0
            for t, rt in enumerate(RS):
                Ft = rt * A
                S6 = 6 * rt
                H = Ft // 2
                src = pav[:, row0 * A:(row0 + rt) * A]
                row0 += rt
                y = io.tile([P, Fmax], f32, tag="y")
                # e/d in bf16: halves SBUF streaming (cross-engine bank
                # contention measurably inflates DVE op durations). x stays
                # fp32 — its quantization error is common-mode across rows and
                # would bias the loss (~1.4e-3); e/d rounding averages out.
                e = wk.tile([P, Fmax], bf16, tag="e")
                d = wk.tile([P, Fmax], bf16, tag="d")
                cums = cm.tile([P, Fmax], f32, tag="cums")
                # x read via 0-stride broadcast AP (no materialized xbb tile).
                # Tile 0 is sliced in quarters (shorter DMA ramp before the
                # first DVE work); later tiles use halves for exp and a single
                # sub on the big tiles.
                if t == 0:
                    Q = Ft // 4
                    ex_sl = sub_sl = tuple(
                        (q * Q, (q + 1) * Q) for q in range(4))
                    nc.sync.dma_start(y[:, :Q], src[:, :Q])
                    nc.sync.dma_start(y[:, Q:H], src[:, Q:H])
                    nc.sync.dma_start(y[:, H:H + Q], src[:, H:H + Q])
                    nc.sync.dma_start(y[:, H + Q:Ft], src[:, H + Q:])
                else:
                    ex_sl = ((0, H), (H, Ft))
                    sub_sl = ex_sl if rt < r_max else ((0, Ft),)
                    nc.sync.dma_start(y[:, :H], src[:, :H])
                    nc.sync.dma_start(y[:, H:Ft], src[:, H:])
                for h0, h1 in ex_sl:
                    nc.scalar.activation(e[:, h0:h1], y[:, h0:h1], Exp)
                for h0, h1 in sub_sl:
                    nc.vector.tensor_tensor(
                        d[:, h0:h1].rearrange("p (r a) -> p r a", a=A),
                        y[:, h0:h1].rearrange("p (r a) -> p r a", a=A),
                        xbt[:, :A].unsqueeze(1).broadcast_to(
                            (P, (h1 - h0) // A, A)),
                        op=sub_op)
                e3 = e[:, :Ft].rearrange("p (r a) -> p r a", r=rt)
                d3 = d[:, :Ft].rearrange("p (r a) -> p r a", r=rt)
                # Z before the cumsum chain: the reduces only need e, and the
                # earlier Z lands, the earlier ScalarE's Ln+accum for this tile
                # overlaps DVE work (trims the last tile's tail).
                # Z double-buffered: ScalarE's Ln reads Z(t), so a single
                # buffer would make tile t+1's Z-reduces wait on ScalarE
                # (cross-engine WAR sems on the DVE queue every tile).
                Z = zp.tile([P, 6 * r_max], f32, tag="Z")
                Z3 = Z[:, :S6].rearrange("p (r s) -> p r s", s=6)
                # Adjacent equal-size segments ({0,1}: n=3, {3,4}: n=25) share
                # one reduce via a 4D [P, r, 2, n] AP — 4 instrs instead of 6.
                # Tile 0 reduces per row-half: each half only needs half of e,
                # so DVE work becomes ready before the slow small-line xbt DMA
                # (which the subs depend on) completes.
                halves = ((0, rt // 2), (rt // 2, rt)) if t == 0 else ((0, rt),)
                for r0, r1 in halves:
                    for j0, k, o, n in ((0, 2, 0, 3), (2, 1, 6, 4),
                                        (3, 2, 10, 25), (5, 1, 60, 8)):
                        nc.vector.tensor_reduce(
                            Z3[:, r0:r1, j0:j0 + k],
                            e3[:, r0:r1, o:o + k * n].rearrange(
                                "p r (k n) -> p r k n", k=k),
                            axis=AX, op=add_op)
                # cums = running sum of e*d over the flat [r*A] stream.
                nc.vector._custom_dve(
                    mul_cumsum, out=cums[:, :Ft], in0=e3, in1=d3)
                cums3 = cums[:, :Ft].rearrange("p (r a) -> p r a", r=rt)
                # Sample the cumsum at each segment-end column; j-innermost
                # layout so one adjacent-difference yields every segment sum.
                # End cols {2,5}, {9,34,59}, {67} have affine strides, so three
                # strided copies cover all six.
                smp = sm.tile([P, 6 * r_max], f32, tag="smp")
                smp3 = smp[:, :S6].rearrange("p (r s) -> p r s", s=6)
                nc.vector.tensor_copy(smp3[:, :, 0:2], cums3[:, :, 2:6:3])
                nc.vector.tensor_copy(smp3[:, :, 2:5], cums3[:, :, 9:60:25])
                nc.vector.tensor_copy(smp3[:, :, 5:6], cums3[:, :, 67:68])
                Sg = sm.tile([P, 6 * r_max], f32, tag="Sg")
                nc.vector.tensor_copy(Sg[:, 0:1], smp[:, 0:1])
                nc.vector.tensor_tensor(
                    Sg[:, 1:S6], smp[:, 1:S6], smp[:, :S6 - 1], op=sub_op)
                rz = sm.tile([P, 6 * r_max], f32, tag="rz")
                nc.vector.reciprocal_approx_fast(rz[:, :S6], Z[:, :S6])
                # Fold inv_n into the reciprocal so one stt covers all 6 segs.
                # inv_n is read via a 0-stride broadcast AP (a materialized
                # copy would be hoisted by the scheduler ahead of tile 0 and
                # stall the DVE queue on the slow small-line xbt DMA).
                rzi = sm.tile([P, 6 * r_max], f32, tag="rzi")
                nc.vector.tensor_tensor(
                    rzi[:, :S6].rearrange("p (r s) -> p r s", s=6),
                    rz[:, :S6].rearrange("p (r s) -> p r s", s=6),
                    xbt[:, A:A + 6].unsqueeze(1).broadcast_to((P, rt, 6)),
                    op=mult_op)
                # Ln accumulation grouped by equal inv_n (segments {0,1}, {2},
                # {3,4}, {5}) — 4 ScalarE instrs instead of 6.
                L = sm.tile([P, 6 * r_max], f32, tag="L")
                for g, (j0, k) in enumerate(((0, 2), (2, 1), (3, 2), (5, 1))):
                    nc.scalar.activation(
                        L[:, j0 * rt:(j0 + k) * rt].rearrange(
                            "p (r s) -> p r s", s=k),
                        Z3[:, :, j0:j0 + k], Ln,
                        accum_out=accB[:, t * 4 + g: t * 4 + g + 1])
                to = sm.tile([P, 6 * r_max], f32, tag="to")
                nc.vector.scalar_tensor_tensor(
                    out=to[:, :S6],
                    in0=Sg[:, :S6],
                    scalar=1.0,
                    in1=rzi[:, :S6],
                    op0=mult_op,
                    op1=mult_op,
                    accum_out=accA[:, t: t + 1])
            nc.sync.dma_start(acc_a[:], accA[:])
            nc.sync.dma_start(acc_b[:], accB[:])
    with _force_exp_ln_one_table_set():
        nc.compile()
    return nc, Tt


def _force_exp_ln_one_table_set():
    """Make the act-table pass map both Exp and Ln to
    natural_log_exp_and_others (otherwise it alternates exp_and_others /
    natural_log per tile: 14 ACT_TABLE_LOADs ~= 18us of ScalarE time)."""
    import contextlib
    import concourse.bacc as bacc_mod
    import concourse.mybir as mybir

    @contextlib.contextmanager
    def ctx():
        orig = bacc_mod.get_activation_tables

        def patched(arch):
            tables = {k: set(v) for k, v in orig(arch).items()}
            for name, funcs in tables.items():
                if name != "natural_log_exp_and_others":
                    funcs.discard(mybir.ActivationFunctionType.Exp)
                    funcs.discard(mybir.ActivationFunctionType.Ln)
            return tables

        bacc_mod.get_activation_tables = patched
        try:
            yield
        finally:
            bacc_mod.get_activation_tables = orig

    return ctx()


def _get_program():
    key = (W_CORE, R)
    if key not in _PROGRAM_CACHE:
        _PROGRAM_CACHE[key] = build_program(W_CORE, R)
    return _PROGRAM_CACHE[key]


def _host_x(current_action):
    """Segmented log_softmax of current_action in float64 on host."""
    ca = np.asarray(current_action, np.float64)
    x = np.empty(A, np.float64)
    for o, n in zip(OFFS, NVEC):
        seg = ca[o:o + n]
        m = seg.max()
        x[o:o + n] = seg - (m + np.log(np.exp(seg - m).sum()))
    return x


def combine_partials(results, w_full=W_FULL):
    """Combine per-core acc_a [P,T] (inv_n-weighted S/Z partials) and
    acc_b [P,T*4] (per-inv_n-group log-sums) into the scalar loss."""
    inv_g = np.asarray([1.0 / 3, 1.0 / 4, 1.0 / 25, 1.0 / 8], np.float64)
    total = 0.0
    for res in results:
        a = np.asarray(res["acc_a"], np.float64)
        b = np.asarray(res["acc_b"], np.float64)
        total += a.sum()  # inv_n already folded in on-device
        bt = b.reshape(P, -1, 4).sum(axis=(0, 1))  # [4] group log-sums
        total -= (bt * inv_g).sum()
    return np.float32(total / w_full)


def _make_xbt(current_action):
    """Host-side xb payload: x broadcast [P, 68] ++ inv_n [P, 6]."""
    x = _host_x(current_action).astype(np.float32)
    row = np.concatenate([x, (1.0 / np.asarray(NVEC, np.float32))])
    return np.broadcast_to(row, (P, A + 6)).copy()


def kernel(current_action, previous_actions):
    from concourse import bass_utils

    nc, _ = _get_program()
    xbt = _make_xbt(current_action)
    pa = np.ascontiguousarray(np.asarray(previous_actions, np.float32))
    assert pa.shape == (W_FULL, A)
    in_maps = [
        {"pa": pa[c * W_CORE:(c + 1) * W_CORE], "xb": xbt}
        for c in range(N_CORES)
    ]
    res = bass_utils.run_bass_kernel_spmd(
        nc, in_maps, core_ids=list(range(N_CORES)))
    return combine_partials(res.results)


if __name__ == "__main__":
    np.random.seed(0)
    ca = np.random.randn(A).astype(np.float32)
    pa = np.random.randn(W_FULL, A).astype(np.float32)
    print(kernel(ca, pa))



# revision 8
# speedup vs baseline: 1.0387x; 1.0387x over previous
"""Trainium2 Bass kernel for nn_ActionSmoothingLoss (v2: 2x DVE scans).

Math (per row y of previous_actions, x = segmented log_softmax(current_action)):
    e = exp(y);  d = y - x
    S_j = sum_{i in seg j} e_i d_i;  Z_j = sum_{i in seg j} e_i
    loss = (1/W) * sum_rows sum_j (1/n_j) * [ S_j / Z_j - log Z_j ]

v2 strategy (baseline v1 = 147.7us, DVE-bound with 3 full 1x passes):
  All three full-tile DVE passes run at 2 elem/cycle (2X_1PORT perf mode):
    - d = y16 - xrep   : builtin tensor_tensor, fp16 in/out (firmware has a
                         2x uop for TT) -- needs y in fp16, which ScalarE
                         produces with an extra Copy pass (ScalarE has slack).
    - cA = cumsum(e*d - m)  : custom DVE op MUL_CUMSUM_C2_ANT with a
                         HAND-AUTHORED 2x uop program (the repo's custom-DVE
                         framework ships 1x only -- "T1" in its design doc;
                         the table-gen side already supports uops_2x).
    - cB = cumsum(e - mz)   : custom CUMSUM_C2_ANT, also 2x. rd1 is force-
                         enabled (dummy in1=e) so the perf-mode byte is
                         TwoSrc and the un-authorable 2-port modes are
                         unreachable.
  2x requires 16-bit in AND out.  fp16 output of a raw cumsum would be
  numerically dead (c grows to ~1e4), so the scans are RECENTERED: the body
  subtracts a host-computed drift constant per element (m ~ E[e*d], mz =
  E[e] = sqrt(e)); the running sum stays O(100) and fp16 sampling error
  telescopes out of the final sum (numpy model: rel err ~3e-6).  The exact
  algebra S_j = diff_j + m*n_j holds for ANY m, so this is not an
  approximation -- m only controls rounding magnitude.  m depends on x, so
  it rides in xbt and reaches the op as a [P,1] AP scalar; mz is static.
  Z_j = diffB_j + mz*n_j is applied by 4 per-inv_n-group tensor_scalar adds
  (fp32 out, feeds reciprocal_approx_fast and ScalarE Ln+accum); the S
  correction folds into scalar_tensor_tensor's op0 (scalar = m*n_g AP).
  Sampling: cums viewed [P, 68, rt]; segment-end cols {2,5},{9,34,59},{67}
  -> smp [P, 6, rt] (j-major) in 3 strided copies; adjacent diff
  smp[1:6]-smp[0:5] is contiguous fp16 -> one 2x sub; the j=0 column uses
  the previous row's last sample (shifted by one) + first-element copy.
Partial sums accA/accB [P, T*4] (per inv_n group); host combines in f64
(inv_n applied host-side).
"""

import sys

sys.path.insert(0, "/opt/trn_rl_repo")

import numpy as np

NVEC = (3, 3, 4, 25, 25, 8)
OFFS = (0, 3, 6, 10, 35, 60)
ENDS = (2, 5, 9, 34, 59, 67)  # inclusive end column of each segment
# inv_n groups: segments {0,1} n=3, {2} n=4, {3,4} n=25, {5} n=8
GRP = ((0, 2), (2, 1), (3, 2), (5, 1))  # (first seg j, count) per group
GRP_N = (3, 4, 25, 8)
A = 68
P = 128
N_CORES = 8
W_FULL = 524288
W_CORE = W_FULL // N_CORES  # 65536
R = 64                      # base rows-per-partition unit for tile schedule
MZ = float(np.exp(0.5))     # E[exp(y)] for y~N(0,1): recenter const of cB

_PROGRAM_CACHE = {}
_OPS = None


def _fsm(seed, steady, step):
    """Wire the 3-state per-page machine: SEED -(count 1)-> STEADY; STEADY
    exits on SRC_TENSOR_DONE, jumps to STEP on SUB_DIM_DONE (page = one row
    of 68); STEP re-seeds the recurrence for one element(pair) then returns.
    Mirrors lower()'s segmented-scan machine / tensor_paged_mask firmware."""
    from concourse.dve_uop import Trigger
    seed.trigger = (Trigger.COUNT, Trigger.NONE, Trigger.NONE)
    seed.repeat_count = 1
    seed.next_uop = (1, 0, 0)
    steady.trigger = (Trigger.SRC_TENSOR_DONE, Trigger.SUB_DIM_DONE,
                      Trigger.NONE)
    steady.next_uop = (0, 2, 0)
    step.trigger = (Trigger.SRC_TENSOR_DONE, Trigger.SUB_DIM_DONE,
                    Trigger.COUNT)
    step.next_uop = (0, 2, 1)
    step.repeat_count = 1
    return [seed, steady, step]


def _build_mul_cumsum_1x():
    """1x 3-uop program for per-row-reset scan(ADD, Src0*Src1 - C0)."""
    import copy
    from concourse.dve_uop import (
        ENABLE, AluInp, AluOp, InpSel, OutPath, OutSel, UopConfig,
    )

    seed = UopConfig()
    seed.enable_input(InpSel.ZERO, 1)
    for b in range(2):
        seed.datapath_config[b].pass_through_delay(0)
    seed.datapath_config[2].enable_alu(AluOp.BYPASS, AluInp.PREV_DELAY_0)

    st = UopConfig()
    st.enable_input(InpSel.SRC_0, 1)               # ch0 = a
    st.enable_input(InpSel.SRC_1, 2)               # ch1 = b
    st.enable_input(InpSel.CONST_0, 3)             # ch2 = m
    blk = st.datapath_config
    blk[0].enable_alu(AluOp.MULTIPLY, AluInp.PREV_DELAY_0, AluInp.PREV_DELAY_1)
    blk[0].pass_through_delay(2)
    blk[1].enable_alu(AluOp.SUBTRACT, AluInp.PREV_ALU_OUT, AluInp.PREV_DELAY_2)
    blk[2].enable_alu(AluOp.ADD, AluInp.CURR_ALU_OUT, AluInp.PREV_ALU_OUT)
    for b in range(3, 8):
        blk[b].pass_through_alu()
    st.enable_output(OutSel.ALU_OUT, OutPath.WR0_LO)
    st.require_inp0 = st.require_inp1 = ENABLE

    step = copy.deepcopy(st)
    step.datapath_config[2].enable_alu(AluOp.BYPASS, AluInp.PREV_ALU_OUT)
    return _fsm(seed, st, step)


def _build_mul_cumsum_2x():
    """2X_1PORT 3-uop program for per-row-reset scan(ADD, Src0*Src1 - C0).

    Per cycle the engine delivers the packed pair (a0,b0),(a1,b1) as
    SRC_0/SRC_1/SRC_0_HI/SRC_1_HI.  Dataflow (one wavefront/cycle):
        p0 = a0*b0 ; p1 = a1*b1 ; s = p0+p1 ; s2 = s - C1   (C1 MUST be 2*C0)
        acc_hi = acc_hi' + s2        (1-cycle recurrence on block 4)
        acc_lo = acc_hi - p1 + C0
    WR0_LO <- acc_lo (elem 2i), WR0_HI <- acc_hi (elem 2i+1).  STEP resets
    the recurrence (acc_hi = s2) for the first pair of each 68-col row."""
    import copy
    from concourse.dve_uop import (
        ENABLE, AluInp, AluOp, DelayInp, InpSel, OutPath, OutSel, UopConfig,
    )

    seed = UopConfig()
    seed.enable_input(InpSel.ZERO, 1)              # chain0 = 0
    for b in range(4):
        seed.datapath_config[b].pass_through_delay(0)
    seed.datapath_config[4].enable_alu(AluOp.BYPASS, AluInp.PREV_DELAY_0)

    st = UopConfig()
    st.enable_input(InpSel.SRC_0, 1)               # ch0 = a0
    st.enable_input(InpSel.SRC_1, 2)               # ch1 = b0
    st.enable_input(InpSel.SRC_0_HI, 3)            # ch2 = a1
    st.enable_input(InpSel.SRC_1_HI, 4)            # ch3 = b1
    st.enable_input(InpSel.CONST_1, 5)             # ch4 = C1 = 2m
    st.enable_input(InpSel.CONST_0, 6)             # ch5 = C0 = m
    blk = st.datapath_config
    blk[0].enable_alu(AluOp.MULTIPLY, AluInp.PREV_DELAY_0, AluInp.PREV_DELAY_1)
    blk[0].pass_through_delay(2, 3, 4, 5)
    blk[1].enable_alu(AluOp.MULTIPLY, AluInp.PREV_DELAY_2, AluInp.PREV_DELAY_3)
    blk[1].enable_delay_from_src(DelayInp.PREV_ALU_OUT, 0)   # ch0 <- p0
    blk[1].pass_through_delay(4, 5)
    blk[2].enable_alu(AluOp.ADD, AluInp.PREV_ALU_OUT, AluInp.PREV_DELAY_0)
    blk[2].enable_delay_from_src(DelayInp.PREV_ALU_OUT, 1)   # ch1 <- p1
    blk[2].pass_through_delay(4, 5)
    blk[3].enable_alu(AluOp.SUBTRACT, AluInp.PREV_ALU_OUT, AluInp.PREV_DELAY_4)
    blk[3].pass_through_delay(1, 5)
    blk[4].enable_alu(AluOp.ADD, AluInp.CURR_ALU_OUT, AluInp.PREV_ALU_OUT)
    blk[4].pass_through_delay(1, 5)
    blk[5].enable_alu(AluOp.SUBTRACT, AluInp.PREV_ALU_OUT, AluInp.PREV_DELAY_1)
    blk[5].enable_delay_from_src(DelayInp.PREV_ALU_OUT, 0)   # ch0 <- acc_hi
    blk[5].pass_through_delay(5)
    blk[6].enable_alu(AluOp.ADD, AluInp.PREV_ALU_OUT, AluInp.PREV_DELAY_5)
    blk[6].pass_through_delay(0)
    blk[7].pass_through_alu()
    blk[7].pass_through_delay(0)
    st.enable_output(OutSel.ALU_OUT, OutPath.WR0_LO)
    st.enable_output(OutSel.DELAY_0, OutPath.WR0_HI)
    st.require_inp0 = st.require_inp1 = ENABLE

    step = copy.deepcopy(st)
    step.datapath_config[4].enable_alu(AluOp.BYPASS, AluInp.PREV_ALU_OUT)
    return _fsm(seed, st, step)


def _build_cumsum_1x():
    """1x 3-uop program for per-row-reset scan(ADD, Src0 - C0); CONSUMES a
    dummy src1 (rd1_en forced on so the perf-mode byte reads TwoSrc and the
    un-authored 2-port modes are unreachable)."""
    import copy
    from concourse.dve_uop import (
        ENABLE, AluInp, AluOp, InpSel, OutPath, OutSel, UopConfig,
    )

    seed = UopConfig()
    seed.enable_input(InpSel.ZERO, 1)
    seed.datapath_config[0].pass_through_delay(0)
    seed.datapath_config[1].enable_alu(AluOp.BYPASS, AluInp.PREV_DELAY_0)

    st = UopConfig()
    st.enable_input(InpSel.SRC_0, 1)               # ch0 = a
    st.enable_input(InpSel.CONST_0, 2)             # ch1 = m
    blk = st.datapath_config
    blk[0].enable_alu(AluOp.SUBTRACT, AluInp.PREV_DELAY_0, AluInp.PREV_DELAY_1)
    blk[1].enable_alu(AluOp.ADD, AluInp.CURR_ALU_OUT, AluInp.PREV_ALU_OUT)
    for b in range(2, 8):
        blk[b].pass_through_alu()
    st.enable_output(OutSel.ALU_OUT, OutPath.WR0_LO)
    st.require_inp0 = st.require_inp1 = ENABLE

    step = copy.deepcopy(st)
    step.datapath_config[1].enable_alu(AluOp.BYPASS, AluInp.PREV_ALU_OUT)
    return _fsm(seed, st, step)


def _build_cumsum_2x():
    """2X_1PORT 3-uop program for per-row-reset scan(ADD, Src0 - C0); src1
    consumed but unread.
        s = a0 + a1 ; s2 = s - C1 (=2m) ; acc_hi = acc_hi' + s2  (block 2)
        acc_lo = acc_hi - a1 + C0"""
    import copy
    from concourse.dve_uop import (
        ENABLE, AluInp, AluOp, DelayInp, InpSel, OutPath, OutSel, UopConfig,
    )

    seed = UopConfig()
    seed.enable_input(InpSel.ZERO, 1)
    for b in range(2):
        seed.datapath_config[b].pass_through_delay(0)
    seed.datapath_config[2].enable_alu(AluOp.BYPASS, AluInp.PREV_DELAY_0)

    st = UopConfig()
    st.enable_input(InpSel.SRC_0, 1)               # ch0 = a0
    st.enable_input(InpSel.SRC_0_HI, 2)            # ch1 = a1
    st.enable_input(InpSel.CONST_1, 3)             # ch2 = 2m
    st.enable_input(InpSel.CONST_0, 4)             # ch3 = m
    blk = st.datapath_config
    blk[0].enable_alu(AluOp.ADD, AluInp.PREV_DELAY_0, AluInp.PREV_DELAY_1)
    blk[0].pass_through_delay(1, 2, 3)
    blk[1].enable_alu(AluOp.SUBTRACT, AluInp.PREV_ALU_OUT, AluInp.PREV_DELAY_2)
    blk[1].pass_through_delay(1, 3)
    blk[2].enable_alu(AluOp.ADD, AluInp.CURR_ALU_OUT, AluInp.PREV_ALU_OUT)
    blk[2].pass_through_delay(1, 3)
    blk[3].enable_alu(AluOp.SUBTRACT, AluInp.PREV_ALU_OUT, AluInp.PREV_DELAY_1)
    blk[3].enable_delay_from_src(DelayInp.PREV_ALU_OUT, 0)   # ch0 <- acc_hi
    blk[3].pass_through_delay(3)
    blk[4].enable_alu(AluOp.ADD, AluInp.PREV_ALU_OUT, AluInp.PREV_DELAY_3)
    blk[4].pass_through_delay(0)
    for b in range(5, 8):
        blk[b].pass_through_alu()
        blk[b].pass_through_delay(0)
    st.enable_output(OutSel.ALU_OUT, OutPath.WR0_LO)
    st.enable_output(OutSel.DELAY_0, OutPath.WR0_HI)
    st.require_inp0 = st.require_inp1 = ENABLE

    step = copy.deepcopy(st)
    step.datapath_config[2].enable_alu(AluOp.BYPASS, AluInp.PREV_ALU_OUT)
    return _fsm(seed, st, step)


def _register_ops():
    """Register MUL_CUMSUM_C2_ANT / CUMSUM_C2_ANT with 1x (lowered or hand)
    and hand-authored 2x programs; pre-seed the compile cache so table-gen
    ships the 2x entries.  CALLER INVARIANT: s1 must equal 2*s0 (the 2x
    program uses C1 for the pair-sum recenter).  Idempotent."""
    global _OPS
    if _OPS is not None:
        return _OPS
    import concourse.dve_ops as dve_ops_mod
    from concourse.dve_ops import _COMPILE_CACHE
    from concourse.dve_spec import AluOp, C0, Spec, Src0, Src1, scan
    from concourse.dve_uop import DveOpSpec

    def _c0(c0, nd):
        if np.isscalar(c0):
            return np.float32(c0)
        a = np.asarray(c0, np.float32)
        return a.reshape(a.shape[0], *([1] * (nd - 1)))

    def _ref_mc(in0, in1, c0, c1, imm2):
        # in0/out [P, S, N] (paged); in1 flat [P, S*N]; cumsum resets per page
        a0 = np.asarray(in0, np.float32)
        a1 = np.asarray(in1, np.float32).reshape(a0.shape)
        prod = a0 * a1 - _c0(c0, a0.ndim)
        return np.cumsum(prod, axis=-1, dtype=np.float32)

    def _ref_c(in0, in1, c0, c1, imm2):
        a0 = np.asarray(in0, np.float32)
        t = a0 - _c0(c0, a0.ndim)
        return np.cumsum(t, axis=-1, dtype=np.float32)

    out = []
    for name, body_mul, ref in (
        ("MUL_CUMSUM_C2_ANT", True, _ref_mc),
        ("CUMSUM_C2_ANT", False, _ref_c),
    ):
        existing = [op for op in dve_ops_mod.OPS if op.name == name]
        if existing:
            out.append(existing[0])
            continue
        # spec.body documents the elementwise semantics and feeds nothing but
        # the CoreSim reference (the per-row reset lives in the hand uops +
        # reference; lower() is not used).
        if body_mul:
            spec = Spec(body=scan(AluOp.ADD, Src0 * Src1 - C0), reference=ref)
            uops_1x = _build_mul_cumsum_1x()
            uops_2x = _build_mul_cumsum_2x()
        else:
            spec = Spec(body=scan(AluOp.ADD, Src0 - C0), reference=ref)
            uops_1x = _build_cumsum_1x()
            uops_2x = _build_cumsum_2x()
        row = dve_ops_mod._CUSTOM_DVE_ROW_BASE + len(dve_ops_mod.OPS)
        assert row < 0x20
        compiled = DveOpSpec(
            name=name, opcode=row, uops=uops_1x, uops_2x=uops_2x,
            rd1_en=True, perf_max=1,
        )
        for u in uops_1x + uops_2x:
            u.validate("v3")
        shas = {"v3": compiled.sha("v3")}
        op = dve_ops_mod.DveOp(name, spec, subdim=True, uops_sha=shas)
        dve_ops_mod.OPS.append(op)
        dve_ops_mod._SUB_OPCODE_FOR_NAME[name] = row
        dve_ops_mod.CUSTOM_DVE_SPECS[name] = spec
        _COMPILE_CACHE[(name, "v3")] = compiled
        out.append(op)
    _OPS = tuple(out)
    return _OPS


def build_program(w_core=W_CORE, r=R):
    import concourse.bass as bass
    import concourse.bacc as bacc
    import concourse.mybir as mybir
    from concourse import tile

    op_mc, op_c = _register_ops()

    f32 = mybir.dt.float32
    f16 = mybir.dt.float16
    rows_pp = w_core // P
    if rows_pp >= 384 and (rows_pp - 128) % 128 == 0:
        RS = [64, 64] + [128] * ((rows_pp - 128) // 128)
    else:
        RS = [64] * (rows_pp // 64)
    assert sum(RS) == rows_pp
    r_max = max(RS)
    Fmax = r_max * A
    XR = 64 * A  # xrep covers 64 rows; bigger tiles subtract in 64-row chunks
    Tt = len(RS)

    Exp = mybir.ActivationFunctionType.Exp
    Ln = mybir.ActivationFunctionType.Ln
    Copy = mybir.ActivationFunctionType.Copy
    sub_op = mybir.AluOpType.subtract
    add_op = mybir.AluOpType.add
    mult_op = mybir.AluOpType.mult

    nc = bacc.Bacc(None, target_bir_lowering=False)
    pa = nc.dram_tensor("pa", [w_core, A], f32, kind="ExternalInput")
    # xb: cols 0..67 = x (fp32); 68 = m; 69 = 2m; 70..73 = m*n_g per group.
    xb = nc.dram_tensor("xb", [P, A + 6], f32, kind="ExternalInput")
    acc_a = nc.dram_tensor("acc_a", [P, Tt * 4], f32, kind="ExternalOutput")
    acc_b = nc.dram_tensor("acc_b", [P, Tt * 4], f32, kind="ExternalOutput")

    pav = pa.rearrange("(p q) a -> p (q a)", p=P)

    def cdve(op, out, in0, in1, s0, s1):
        inst = nc.vector._custom_dve(op, out=out, in0=in0, in1=in1, s0=s0, s1=s1)
        inst.perf_max = 1
        return inst

    with tile.TileContext(nc) as tc:
        with tc.tile_pool(name="ps", bufs=1) as ps, \
             tc.tile_pool(name="io", bufs=2) as io, \
             tc.tile_pool(name="ep", bufs=2) as ep, \
             tc.tile_pool(name="dp", bufs=1) as dp, \
             tc.tile_pool(name="cm", bufs=1) as cm, \
             tc.tile_pool(name="sm", bufs=1) as sm, \
             tc.tile_pool(name="zp", bufs=2) as zp:
            xbt = ps.tile([P, A + 6], f32)
            nc.sync.dma_start(xbt[:], xb[:], single_packet=True)
            m_ap = xbt[:, A:A + 1]
            m2_ap = xbt[:, A + 1:A + 2]
            mn_ap = [xbt[:, A + 2 + g:A + 3 + g] for g in range(4)]
            # x replicated across 64 rows, fp16 (one-time).
            xrep = ps.tile([P, XR], f16)
            nc.vector.tensor_copy(
                xrep[:].rearrange("p (r a) -> p r a", a=A),
                xbt[:, :A].unsqueeze(1).broadcast_to((P, 64, A)))
            accA = ps.tile([P, Tt * 4], f32)
            accB = ps.tile([P, Tt * 4], f32)
            row0 = 0
            for t, rt in enumerate(RS):
                Ft = rt * A
                S6 = 6 * rt
                H = Ft // 2
                src = pav[:, row0 * A:(row0 + rt) * A]
                row0 += rt
                y = io.tile([P, Fmax], f32, tag="y")
                e = ep.tile([P, Fmax], f16, tag="e")
                d = dp.tile([P, Fmax], f16, tag="d")
                cA = cm.tile([P, Fmax], f16, tag="cA")
                cB = cm.tile([P, Fmax], f16, tag="cB")
                if t == 0:
                    Q = Ft // 4
                    sl = tuple((q * Q, (q + 1) * Q) for q in range(4))
                    for h0, h1 in sl:
                        nc.sync.dma_start(y[:, h0:h1], src[:, h0:h1])
                else:
                    sl = ((0, H), (H, Ft))
                    nc.sync.dma_start(y[:, :H], src[:, :H])
                    nc.sync.dma_start(y[:, H:Ft], src[:, H:])
                # ScalarE: e = exp(y) fp16; y16 = Copy(y) fp16 (into d; the
                # DVE subtract then runs in-place at 2x).
                for h0, h1 in sl:
                    nc.scalar.activation(e[:, h0:h1], y[:, h0:h1], Exp)
                for h0, h1 in sl:
                    nc.scalar.activation(d[:, h0:h1], y[:, h0:h1], Copy)
                # DVE big passes (all 2X_1PORT). Scans are per-row-reset:
                # in0/out are [P, rt, 68] paged APs (subdim ops); in1 rides
                # flat (TTSS struct, so C1 can be a [P,1] AP).
                e3 = e[:, :Ft].rearrange("p (r a) -> p r a", a=A)
                cdve(op_c, cB[:, :Ft].rearrange("p (r a) -> p r a", a=A),
                     e3, e[:, :Ft], MZ, 2.0 * MZ)
                for c0 in range(0, Ft, XR):
                    c1 = min(c0 + XR, Ft)
                    nc.vector.tensor_tensor(
                        d[:, c0:c1], d[:, c0:c1], xrep[:, :c1 - c0], op=sub_op)
                cdve(op_mc, cA[:, :Ft].rearrange("p (r a) -> p r a", a=A),
                     e3, d[:, :Ft], m_ap, m2_ap)
                # Sampling: c viewed [P, a, r]; 3 strided copies -> [P, 6, rt]
                # j-major; adjacent diff of j=1..5 is one contiguous 2x sub.
                smpA = sm.tile([P, 6 * r_max], f16, tag="smpA")
                smpB = sm.tile([P, 6 * r_max], f16, tag="smpB")
                SgA = sm.tile([P, 6 * r_max], f16, tag="SgA")
                SgB = sm.tile([P, 6 * r_max], f16, tag="SgB")
                for cums, smp in ((cB, smpB), (cA, smpA)):
                    cT = cums[:, :Ft].rearrange("p (r a) -> p a r", a=A)
                    smp3 = smp[:, :S6].rearrange("p (j r) -> p j r", j=6)
                    nc.vector.tensor_copy(smp3[:, 0:2], cT[:, 2:6:3])
                    nc.vector.tensor_copy(smp3[:, 2:5], cT[:, 9:60:25])
                    nc.vector.tensor_copy(smp3[:, 5:6], cT[:, 67:68])
                for smp, Sg in ((smpB, SgB), (smpA, SgA)):
                    # scan resets per row, so S_0 = smp0 directly.
                    nc.vector.tensor_copy(Sg[:, 0:rt], smp[:, 0:rt])
                    nc.vector.tensor_tensor(
                        Sg[:, rt:S6], smp[:, rt:S6], smp[:, :5 * rt], op=sub_op)
                # Z = SgB + mz*n_j (per-group const): fp32 out for recip/Ln.
                Zc = zp.tile([P, 6 * r_max], f32, tag="Zc")
                for g, (j0, k) in enumerate(GRP):
                    nc.vector.tensor_scalar(
                        Zc[:, j0 * rt:(j0 + k) * rt],
                        SgB[:, j0 * rt:(j0 + k) * rt],
                        float(MZ * GRP_N[g]), None, op0=add_op)
                rz = sm.tile([P, 6 * r_max], f32, tag="rz")
                nc.vector.reciprocal_approx_fast(rz[:, :S6], Zc[:, :S6])
                to = sm.tile([P, 6 * r_max], f32, tag="to")
                L = sm.tile([P, 6 * r_max], f16, tag="L")
                for g, (j0, k) in enumerate(GRP):
                    sl6 = slice(j0 * rt, (j0 + k) * rt)
                    # accA_g += sum (SgA + m*n_g) * (1/Z)
                    nc.vector.scalar_tensor_tensor(
                        out=to[:, sl6], in0=SgA[:, sl6], scalar=mn_ap[g],
                        in1=rz[:, sl6], op0=add_op, op1=mult_op,
                        accum_out=accA[:, t * 4 + g:t * 4 + g + 1])
                    nc.scalar.activation(
                        L[:, sl6], Zc[:, sl6], Ln,
                        accum_out=accB[:, t * 4 + g:t * 4 + g + 1])
            nc.sync.dma_start(acc_a[:], accA[:])
            nc.sync.dma_start(acc_b[:], accB[:])
    with _force_exp_ln_one_table_set():
        nc.compile()
    return nc, Tt


def _force_exp_ln_one_table_set():
    """Map Exp and Ln (and Copy, which the set already contains) to the single
    natural_log_exp_and_others table so ScalarE never reloads act tables."""
    import contextlib
    import concourse.bacc as bacc_mod
    import concourse.mybir as mybir

    @contextlib.contextmanager
    def ctx():
        orig = bacc_mod.get_activation_tables

        def patched(arch):
            tables = {k: set(v) for k, v in orig(arch).items()}
            for name, funcs in tables.items():
                if name != "natural_log_exp_and_others":
                    funcs.discard(mybir.ActivationFunctionType.Exp)
                    funcs.discard(mybir.ActivationFunctionType.Ln)
            return tables

        bacc_mod.get_activation_tables = patched
        try:
            yield
        finally:
            bacc_mod.get_activation_tables = orig

    return ctx()


def _get_program():
    key = (W_CORE, R)
    if key not in _PROGRAM_CACHE:
        _PROGRAM_CACHE[key] = build_program(W_CORE, R)
    return _PROGRAM_CACHE[key]


def _host_x(current_action):
    """Segmented log_softmax of current_action in float64 on host."""
    ca = np.asarray(current_action, np.float64)
    x = np.empty(A, np.float64)
    for o, n in zip(OFFS, NVEC):
        seg = ca[o:o + n]
        mx = seg.max()
        x[o:o + n] = seg - (mx + np.log(np.exp(seg - mx).sum()))
    return x


def _x_corr(x):
    """Expected bias from the fp16 quantization of x, removed host-side.

    The device computes d with x16 = fp16(x); the excess in the loss is
    sum_rows sum_j inv_n_j * sum_{i in j} w_i * (x_i - x16_i) with softmax
    weights w.  E[w_i] = 1/n_j for iid inputs, so the expected excess per
    row is sum_j (1/n_j^2) * sum_{i in j} delta_i (exact algebra otherwise
    untouched; residual is O(1e-5))."""
    delta = np.asarray(x, np.float64) - np.asarray(x, np.float32).astype(
        np.float16).astype(np.float64)
    return sum((1.0 / (n * n)) * delta[o:o + n].sum()
               for o, n in zip(OFFS, NVEC))


def combine_partials(results, w_full=W_FULL, x_corr=0.0):
    """accA = per-group sums of S/Z; accB = per-group sums of ln Z.
    loss = (1/W) * sum_g inv_n_g * (accA_g - accB_g) - x_corr."""
    inv_g = np.asarray([1.0 / 3, 1.0 / 4, 1.0 / 25, 1.0 / 8], np.float64)
    total = 0.0
    for res in results:
        a = np.asarray(res["acc_a"], np.float64).reshape(P, -1, 4).sum((0, 1))
        b = np.asarray(res["acc_b"], np.float64).reshape(P, -1, 4).sum((0, 1))
        total += (inv_g * (a - b)).sum()
    return np.float32(total / w_full - x_corr)


def _make_xbt(current_action):
    """xb payload: x (68) ++ m ++ 2m ++ m*n_g (4), broadcast to P rows."""
    x = _host_x(current_action)
    m = float(np.exp(0.5) * (1.0 - x.mean()))
    row = np.concatenate([
        x.astype(np.float32),
        np.asarray([m, 2 * m] + [m * n for n in GRP_N], np.float32)])
    return np.broadcast_to(row, (P, A + 6)).copy()


def kernel(current_action, previous_actions):
    from concourse import bass_utils

    nc, _ = _get_program()
    xbt = _make_xbt(current_action)
    pa = np.ascontiguousarray(np.asarray(previous_actions, np.float32))
    assert pa.shape == (W_FULL, A)
    in_maps = [
        {"pa": pa[c * W_CORE:(c + 1) * W_CORE], "xb": xbt}
        for c in range(N_CORES)
    ]
    res = bass_utils.run_bass_kernel_spmd(
        nc, in_maps, core_ids=list(range(N_CORES)))
    return combine_partials(
        res.results, x_corr=_x_corr(_host_x(current_action)))


if __name__ == "__main__":
    np.random.seed(0)
    ca = np.random.randn(A).astype(np.float32)
    pa = np.random.randn(W_FULL, A).astype(np.float32)
    print(kernel(ca, pa))


# revision 9
# speedup vs baseline: 1.3418x; 1.2917x over previous
"""Trainium2 Bass kernel for nn_ActionSmoothingLoss (v2: 2x DVE scans).

Math (per row y of previous_actions, x = segmented log_softmax(current_action)):
    e = exp(y);  d = y - x
    S_j = sum_{i in seg j} e_i d_i;  Z_j = sum_{i in seg j} e_i
    loss = (1/W) * sum_rows sum_j (1/n_j) * [ S_j / Z_j - log Z_j ]

v2 strategy (baseline v1 = 147.7us, DVE-bound with 3 full 1x passes):
  All three full-tile DVE passes run at 2 elem/cycle (2X_1PORT perf mode):
    - d = y16 - xrep   : builtin tensor_tensor, fp16 in/out (firmware has a
                         2x uop for TT) -- needs y in fp16, which ScalarE
                         produces with an extra Copy pass (ScalarE has slack).
    - cA = cumsum(e*d - m)  : custom DVE op MUL_CUMSUM_C2_ANT with a
                         HAND-AUTHORED 2x uop program (the repo's custom-DVE
                         framework ships 1x only -- "T1" in its design doc;
                         the table-gen side already supports uops_2x).
    - cB = cumsum(e - mz)   : custom CUMSUM_C2_ANT, also 2x. rd1 is force-
                         enabled (dummy in1=e) so the perf-mode byte is
                         TwoSrc and the un-authorable 2-port modes are
                         unreachable.
  2x requires 16-bit in AND out.  fp16 output of a raw cumsum would be
  numerically dead (c grows to ~1e4), so the scans are RECENTERED: the body
  subtracts a host-computed drift constant per element (m ~ E[e*d], mz =
  E[e] = sqrt(e)); the running sum stays O(100) and fp16 sampling error
  telescopes out of the final sum (numpy model: rel err ~3e-6).  The exact
  algebra S_j = diff_j + m*n_j holds for ANY m, so this is not an
  approximation -- m only controls rounding magnitude.  m depends on x, so
  it rides in xbt and reaches the op as a [P,1] AP scalar; mz is static.
  Z_j = diffB_j + mz*n_j is applied by 4 per-inv_n-group tensor_scalar adds
  (fp32 out, feeds reciprocal_approx_fast and ScalarE Ln+accum); the S
  correction folds into scalar_tensor_tensor's op0 (scalar = m*n_g AP).
  Sampling: cums viewed [P, 68, rt]; segment-end cols {2,5},{9,34,59},{67}
  -> smp [P, 6, rt] (j-major) in 3 strided copies; adjacent diff
  smp[1:6]-smp[0:5] is contiguous fp16 -> one 2x sub; the j=0 column uses
  the previous row's last sample (shifted by one) + first-element copy.
Partial sums accA/accB [P, T*4] (per inv_n group); host combines in f64
(inv_n applied host-side).
"""

import sys

sys.path.insert(0, "/opt/trn_rl_repo")

import numpy as np

NVEC = (3, 3, 4, 25, 25, 8)
OFFS = (0, 3, 6, 10, 35, 60)
ENDS = (2, 5, 9, 34, 59, 67)  # inclusive end column of each segment
# inv_n groups: segments {0,1} n=3, {2} n=4, {3,4} n=25, {5} n=8
GRP = ((0, 2), (2, 1), (3, 2), (5, 1))  # (first seg j, count) per group
GRP_N = (3, 4, 25, 8)
A = 68
P = 128
N_CORES = 8
W_FULL = 524288
W_CORE = W_FULL // N_CORES  # 65536
R = 64                      # base rows-per-partition unit for tile schedule
MZ = float(np.exp(0.5))     # E[exp(y)] for y~N(0,1): recenter const of cB

_PROGRAM_CACHE = {}
_OPS = None


def _fsm(seed, steady, step):
    """Wire the 3-state per-page machine: SEED -(count 1)-> STEADY; STEADY
    exits on SRC_TENSOR_DONE, jumps to STEP on SUB_DIM_DONE (page = one row
    of 68); STEP re-seeds the recurrence for one element(pair) then returns.
    Mirrors lower()'s segmented-scan machine / tensor_paged_mask firmware."""
    from concourse.dve_uop import Trigger
    seed.trigger = (Trigger.COUNT, Trigger.NONE, Trigger.NONE)
    seed.repeat_count = 1
    seed.next_uop = (1, 0, 0)
    steady.trigger = (Trigger.SRC_TENSOR_DONE, Trigger.SUB_DIM_DONE,
                      Trigger.NONE)
    steady.next_uop = (0, 2, 0)
    step.trigger = (Trigger.SRC_TENSOR_DONE, Trigger.SUB_DIM_DONE,
                    Trigger.COUNT)
    step.next_uop = (0, 2, 1)
    step.repeat_count = 1
    return [seed, steady, step]


def _build_mul_cumsum_1x():
    """1x 3-uop program for per-row-reset scan(ADD, Src0*Src1 - C0)."""
    import copy
    from concourse.dve_uop import (
        ENABLE, AluInp, AluOp, InpSel, OutPath, OutSel, UopConfig,
    )

    seed = UopConfig()
    seed.enable_input(InpSel.ZERO, 1)
    for b in range(2):
        seed.datapath_config[b].pass_through_delay(0)
    seed.datapath_config[2].enable_alu(AluOp.BYPASS, AluInp.PREV_DELAY_0)

    st = UopConfig()
    st.enable_input(InpSel.SRC_0, 1)               # ch0 = a
    st.enable_input(InpSel.SRC_1, 2)               # ch1 = b
    st.enable_input(InpSel.CONST_0, 3)             # ch2 = m
    blk = st.datapath_config
    blk[0].enable_alu(AluOp.MULTIPLY, AluInp.PREV_DELAY_0, AluInp.PREV_DELAY_1)
    blk[0].pass_through_delay(2)
    blk[1].enable_alu(AluOp.SUBTRACT, AluInp.PREV_ALU_OUT, AluInp.PREV_DELAY_2)
    blk[2].enable_alu(AluOp.ADD, AluInp.CURR_ALU_OUT, AluInp.PREV_ALU_OUT)
    for b in range(3, 8):
        blk[b].pass_through_alu()
    st.enable_output(OutSel.ALU_OUT, OutPath.WR0_LO)
    st.require_inp0 = st.require_inp1 = ENABLE

    step = copy.deepcopy(st)
    step.datapath_config[2].enable_alu(AluOp.BYPASS, AluInp.PREV_ALU_OUT)
    return _fsm(seed, st, step)


def _build_mul_cumsum_2x():
    """2X_1PORT 3-uop program for per-row-reset scan(ADD, Src0*Src1 - C0).

    Per cycle the engine delivers the packed pair (a0,b0),(a1,b1) as
    SRC_0/SRC_1/SRC_0_HI/SRC_1_HI.  Dataflow (one wavefront/cycle):
        p0 = a0*b0 ; p1 = a1*b1 ; s = p0+p1 ; s2 = s - C1   (C1 MUST be 2*C0)
        acc_hi = acc_hi' + s2        (1-cycle recurrence on block 4)
        acc_lo = acc_hi - p1 + C0
    WR0_LO <- acc_lo (elem 2i), WR0_HI <- acc_hi (elem 2i+1).  STEP resets
    the recurrence (acc_hi = s2) for the first pair of each 68-col row."""
    import copy
    from concourse.dve_uop import (
        ENABLE, AluInp, AluOp, DelayInp, InpSel, OutPath, OutSel, UopConfig,
    )

    seed = UopConfig()
    seed.enable_input(InpSel.ZERO, 1)              # chain0 = 0
    for b in range(4):
        seed.datapath_config[b].pass_through_delay(0)
    seed.datapath_config[4].enable_alu(AluOp.BYPASS, AluInp.PREV_DELAY_0)

    st = UopConfig()
    st.enable_input(InpSel.SRC_0, 1)               # ch0 = a0
    st.enable_input(InpSel.SRC_1, 2)               # ch1 = b0
    st.enable_input(InpSel.SRC_0_HI, 3)            # ch2 = a1
    st.enable_input(InpSel.SRC_1_HI, 4)            # ch3 = b1
    st.enable_input(InpSel.CONST_1, 5)             # ch4 = C1 = 2m
    st.enable_input(InpSel.CONST_0, 6)             # ch5 = C0 = m
    blk = st.datapath_config
    blk[0].enable_alu(AluOp.MULTIPLY, AluInp.PREV_DELAY_0, AluInp.PREV_DELAY_1)
    blk[0].pass_through_delay(2, 3, 4, 5)
    blk[1].enable_alu(AluOp.MULTIPLY, AluInp.PREV_DELAY_2, AluInp.PREV_DELAY_3)
    blk[1].enable_delay_from_src(DelayInp.PREV_ALU_OUT, 0)   # ch0 <- p0
    blk[1].pass_through_delay(4, 5)
    blk[2].enable_alu(AluOp.ADD, AluInp.PREV_ALU_OUT, AluInp.PREV_DELAY_0)
    blk[2].enable_delay_from_src(DelayInp.PREV_ALU_OUT, 1)   # ch1 <- p1
    blk[2].pass_through_delay(4, 5)
    blk[3].enable_alu(AluOp.SUBTRACT, AluInp.PREV_ALU_OUT, AluInp.PREV_DELAY_4)
    blk[3].pass_through_delay(1, 5)
    blk[4].enable_alu(AluOp.ADD, AluInp.CURR_ALU_OUT, AluInp.PREV_ALU_OUT)
    blk[4].pass_through_delay(1, 5)
    blk[5].enable_alu(AluOp.SUBTRACT, AluInp.PREV_ALU_OUT, AluInp.PREV_DELAY_1)
    blk[5].enable_delay_from_src(DelayInp.PREV_ALU_OUT, 0)   # ch0 <- acc_hi
    blk[5].pass_through_delay(5)
    blk[6].enable_alu(AluOp.ADD, AluInp.PREV_ALU_OUT, AluInp.PREV_DELAY_5)
    blk[6].pass_through_delay(0)
    blk[7].pass_through_alu()
    blk[7].pass_through_delay(0)
    st.enable_output(OutSel.ALU_OUT, OutPath.WR0_LO)
    st.enable_output(OutSel.DELAY_0, OutPath.WR0_HI)
    st.require_inp0 = st.require_inp1 = ENABLE

    step = copy.deepcopy(st)
    step.datapath_config[4].enable_alu(AluOp.BYPASS, AluInp.PREV_ALU_OUT)
    return _fsm(seed, st, step)


def _build_cumsum_1x():
    """1x 3-uop program for per-row-reset scan(ADD, Src0 - C0); CONSUMES a
    dummy src1 (rd1_en forced on so the perf-mode byte reads TwoSrc and the
    un-authored 2-port modes are unreachable)."""
    import copy
    from concourse.dve_uop import (
        ENABLE, AluInp, AluOp, InpSel, OutPath, OutSel, UopConfig,
    )

    seed = UopConfig()
    seed.enable_input(InpSel.ZERO, 1)
    seed.datapath_config[0].pass_through_delay(0)
    seed.datapath_config[1].enable_alu(AluOp.BYPASS, AluInp.PREV_DELAY_0)

    st = UopConfig()
    st.enable_input(InpSel.SRC_0, 1)               # ch0 = a
    st.enable_input(InpSel.CONST_0, 2)             # ch1 = m
    blk = st.datapath_config
    blk[0].enable_alu(AluOp.SUBTRACT, AluInp.PREV_DELAY_0, AluInp.PREV_DELAY_1)
    blk[1].enable_alu(AluOp.ADD, AluInp.CURR_ALU_OUT, AluInp.PREV_ALU_OUT)
    for b in range(2, 8):
        blk[b].pass_through_alu()
    st.enable_output(OutSel.ALU_OUT, OutPath.WR0_LO)
    st.require_inp0 = st.require_inp1 = ENABLE

    step = copy.deepcopy(st)
    step.datapath_config[1].enable_alu(AluOp.BYPASS, AluInp.PREV_ALU_OUT)
    return _fsm(seed, st, step)


def _build_cumsum_2x():
    """2X_1PORT 3-uop program for per-row-reset scan(ADD, Src0 - C0); src1
    consumed but unread.
        s = a0 + a1 ; s2 = s - C1 (=2m) ; acc_hi = acc_hi' + s2  (block 2)
        acc_lo = acc_hi - a1 + C0"""
    import copy
    from concourse.dve_uop import (
        ENABLE, AluInp, AluOp, DelayInp, InpSel, OutPath, OutSel, UopConfig,
    )

    seed = UopConfig()
    seed.enable_input(InpSel.ZERO, 1)
    for b in range(2):
        seed.datapath_config[b].pass_through_delay(0)
    seed.datapath_config[2].enable_alu(AluOp.BYPASS, AluInp.PREV_DELAY_0)

    st = UopConfig()
    st.enable_input(InpSel.SRC_0, 1)               # ch0 = a0
    st.enable_input(InpSel.SRC_0_HI, 2)            # ch1 = a1
    st.enable_input(InpSel.CONST_1, 3)             # ch2 = 2m
    st.enable_input(InpSel.CONST_0, 4)             # ch3 = m
    blk = st.datapath_config
    blk[0].enable_alu(AluOp.ADD, AluInp.PREV_DELAY_0, AluInp.PREV_DELAY_1)
    blk[0].pass_through_delay(1, 2, 3)
    blk[1].enable_alu(AluOp.SUBTRACT, AluInp.PREV_ALU_OUT, AluInp.PREV_DELAY_2)
    blk[1].pass_through_delay(1, 3)
    blk[2].enable_alu(AluOp.ADD, AluInp.CURR_ALU_OUT, AluInp.PREV_ALU_OUT)
    blk[2].pass_through_delay(1, 3)
    blk[3].enable_alu(AluOp.SUBTRACT, AluInp.PREV_ALU_OUT, AluInp.PREV_DELAY_1)
    blk[3].enable_delay_from_src(DelayInp.PREV_ALU_OUT, 0)   # ch0 <- acc_hi
    blk[3].pass_through_delay(3)
    blk[4].enable_alu(AluOp.ADD, AluInp.PREV_ALU_OUT, AluInp.PREV_DELAY_3)
    blk[4].pass_through_delay(0)
    for b in range(5, 8):
        blk[b].pass_through_alu()
        blk[b].pass_through_delay(0)
    st.enable_output(OutSel.ALU_OUT, OutPath.WR0_LO)
    st.enable_output(OutSel.DELAY_0, OutPath.WR0_HI)
    st.require_inp0 = st.require_inp1 = ENABLE

    step = copy.deepcopy(st)
    step.datapath_config[2].enable_alu(AluOp.BYPASS, AluInp.PREV_ALU_OUT)
    return _fsm(seed, st, step)


def _register_ops():
    """Register MUL_CUMSUM_C2_ANT / CUMSUM_C2_ANT with 1x (lowered or hand)
    and hand-authored 2x programs; pre-seed the compile cache so table-gen
    ships the 2x entries.  CALLER INVARIANT: s1 must equal 2*s0 (the 2x
    program uses C1 for the pair-sum recenter).  Idempotent."""
    global _OPS
    if _OPS is not None:
        return _OPS
    import concourse.dve_ops as dve_ops_mod
    from concourse.dve_ops import _COMPILE_CACHE
    from concourse.dve_spec import AluOp, C0, Spec, Src0, Src1, scan
    from concourse.dve_uop import DveOpSpec

    def _c0(c0, nd):
        if np.isscalar(c0):
            return np.float32(c0)
        a = np.asarray(c0, np.float32)
        return a.reshape(a.shape[0], *([1] * (nd - 1)))

    def _ref_mc(in0, in1, c0, c1, imm2):
        # in0/out [P, S, N] (paged); in1 flat [P, S*N]; cumsum resets per page
        a0 = np.asarray(in0, np.float32)
        a1 = np.asarray(in1, np.float32).reshape(a0.shape)
        prod = a0 * a1 - _c0(c0, a0.ndim)
        return np.cumsum(prod, axis=-1, dtype=np.float32)

    def _ref_c(in0, in1, c0, c1, imm2):
        a0 = np.asarray(in0, np.float32)
        t = a0 - _c0(c0, a0.ndim)
        return np.cumsum(t, axis=-1, dtype=np.float32)

    out = []
    for name, body_mul, ref in (
        ("MUL_CUMSUM_C2_ANT", True, _ref_mc),
        ("CUMSUM_C2_ANT", False, _ref_c),
    ):
        existing = [op for op in dve_ops_mod.OPS if op.name == name]
        if existing:
            out.append(existing[0])
            continue
        # spec.body documents the elementwise semantics and feeds nothing but
        # the CoreSim reference (the per-row reset lives in the hand uops +
        # reference; lower() is not used).
        if body_mul:
            spec = Spec(body=scan(AluOp.ADD, Src0 * Src1 - C0), reference=ref)
            uops_1x = _build_mul_cumsum_1x()
            uops_2x = _build_mul_cumsum_2x()
        else:
            spec = Spec(body=scan(AluOp.ADD, Src0 - C0), reference=ref)
            uops_1x = _build_cumsum_1x()
            uops_2x = _build_cumsum_2x()
        row = dve_ops_mod._CUSTOM_DVE_ROW_BASE + len(dve_ops_mod.OPS)
        assert row < 0x20
        compiled = DveOpSpec(
            name=name, opcode=row, uops=uops_1x, uops_2x=uops_2x,
            rd1_en=True, perf_max=1,
        )
        for u in uops_1x + uops_2x:
            u.validate("v3")
        shas = {"v3": compiled.sha("v3")}
        op = dve_ops_mod.DveOp(name, spec, subdim=True, uops_sha=shas)
        dve_ops_mod.OPS.append(op)
        dve_ops_mod._SUB_OPCODE_FOR_NAME[name] = row
        dve_ops_mod.CUSTOM_DVE_SPECS[name] = spec
        _COMPILE_CACHE[(name, "v3")] = compiled
        out.append(op)
    _OPS = tuple(out)
    return _OPS


def build_program(w_core=W_CORE, r=R):
    import concourse.bass as bass
    import concourse.bacc as bacc
    import concourse.mybir as mybir
    from concourse import tile

    op_mc, op_c = _register_ops()

    f32 = mybir.dt.float32
    f16 = mybir.dt.float16
    rows_pp = w_core // P
    if rows_pp >= 384 and (rows_pp - 128) % 128 == 0:
        RS = [64, 64] + [128] * ((rows_pp - 128) // 128)
    else:
        RS = [64] * (rows_pp // 64)
    assert sum(RS) == rows_pp
    r_max = max(RS)
    Fmax = r_max * A
    XR = 64 * A  # xrep covers 64 rows; bigger tiles subtract in 64-row chunks
    Tt = len(RS)

    Exp = mybir.ActivationFunctionType.Exp
    Ln = mybir.ActivationFunctionType.Ln
    Copy = mybir.ActivationFunctionType.Copy
    sub_op = mybir.AluOpType.subtract
    add_op = mybir.AluOpType.add
    mult_op = mybir.AluOpType.mult

    nc = bacc.Bacc(None, target_bir_lowering=False)
    pa = nc.dram_tensor("pa", [w_core, A], f32, kind="ExternalInput")
    # xb: cols 0..67 = x (fp32); 68 = m; 69 = 2m; 70..73 = m*n_g per group.
    xb = nc.dram_tensor("xb", [P, A + 6], f32, kind="ExternalInput")
    acc_a = nc.dram_tensor("acc_a", [P, Tt * 4], f32, kind="ExternalOutput")
    acc_b = nc.dram_tensor("acc_b", [P, Tt * 4], f32, kind="ExternalOutput")

    pav = pa.rearrange("(p q) a -> p (q a)", p=P)

    def cdve(op, out, in0, in1, s0, s1):
        # perf_max must be set at construction (add_instruction copies the
        # instruction into the Rust module; post-hoc mutation is lost), so
        # wrap the class with a kwarg-injecting factory for this emit.
        from concourse import bass_isa as bi
        real = bi.InstCustomDveAnt

        def patched(*a, **kw):
            kw.setdefault("perf_max", 1)
            return real(*a, **kw)

        bi.InstCustomDveAnt = patched
        try:
            return nc.vector._custom_dve(
                op, out=out, in0=in0, in1=in1, s0=s0, s1=s1)
        finally:
            bi.InstCustomDveAnt = real

    with tile.TileContext(nc) as tc:
        with tc.tile_pool(name="ps", bufs=1) as ps, \
             tc.tile_pool(name="io", bufs=2) as io, \
             tc.tile_pool(name="ep", bufs=2) as ep, \
             tc.tile_pool(name="dp", bufs=1) as dp, \
             tc.tile_pool(name="cm", bufs=1) as cm, \
             tc.tile_pool(name="sm", bufs=1) as sm, \
             tc.tile_pool(name="zp", bufs=2) as zp:
            xbt = ps.tile([P, A + 6], f32)
            nc.sync.dma_start(xbt[:], xb[:], single_packet=True)
            m_ap = xbt[:, A:A + 1]
            m2_ap = xbt[:, A + 1:A + 2]
            mn_ap = [xbt[:, A + 2 + g:A + 3 + g] for g in range(4)]
            # x replicated across 64 rows, fp16 (one-time).
            xrep = ps.tile([P, XR], f16)
            nc.vector.tensor_copy(
                xrep[:].rearrange("p (r a) -> p r a", a=A),
                xbt[:, :A].unsqueeze(1).broadcast_to((P, 64, A)))
            accA = ps.tile([P, Tt * 4], f32)
            accB = ps.tile([P, Tt * 4], f32)
            row0 = 0
            for t, rt in enumerate(RS):
                Ft = rt * A
                S6 = 6 * rt
                H = Ft // 2
                src = pav[:, row0 * A:(row0 + rt) * A]
                row0 += rt
                y = io.tile([P, Fmax], f32, tag="y")
                e = ep.tile([P, Fmax], f16, tag="e")
                d = dp.tile([P, Fmax], f16, tag="d")
                cA = cm.tile([P, Fmax], f16, tag="cA")
                cB = cm.tile([P, Fmax], f16, tag="cB")
                if t == 0:
                    Q = Ft // 4
                    sl = tuple((q * Q, (q + 1) * Q) for q in range(4))
                    for h0, h1 in sl:
                        nc.sync.dma_start(y[:, h0:h1], src[:, h0:h1])
                else:
                    sl = ((0, H), (H, Ft))
                    nc.sync.dma_start(y[:, :H], src[:, :H])
                    nc.sync.dma_start(y[:, H:Ft], src[:, H:])
                # ScalarE: e = exp(y) fp16; y16 = Copy(y) fp16 (into d; the
                # DVE subtract then runs in-place at 2x).
                for h0, h1 in sl:
                    nc.scalar.activation(e[:, h0:h1], y[:, h0:h1], Exp)
                for h0, h1 in sl:
                    nc.scalar.activation(d[:, h0:h1], y[:, h0:h1], Copy)
                # DVE big passes (all 2X_1PORT). Scans are per-row-reset:
                # in0/out are [P, rt, 68] paged APs (subdim ops); in1 rides
                # flat (TTSS struct, so C1 can be a [P,1] AP).
                e3 = e[:, :Ft].rearrange("p (r a) -> p r a", a=A)
                cdve(op_c, cB[:, :Ft].rearrange("p (r a) -> p r a", a=A),
                     e3, e[:, :Ft], MZ, 2.0 * MZ)
                for c0 in range(0, Ft, XR):
                    c1 = min(c0 + XR, Ft)
                    nc.vector.tensor_tensor(
                        d[:, c0:c1], d[:, c0:c1], xrep[:, :c1 - c0], op=sub_op)
                cdve(op_mc, cA[:, :Ft].rearrange("p (r a) -> p r a", a=A),
                     e3, d[:, :Ft], m_ap, m2_ap)
                # Sampling: c viewed [P, a, r]; 3 strided copies -> [P, 6, rt]
                # j-major; adjacent diff of j=1..5 is one contiguous 2x sub.
                smpA = sm.tile([P, 6 * r_max], f16, tag="smpA")
                smpB = sm.tile([P, 6 * r_max], f16, tag="smpB")
                SgA = sm.tile([P, 6 * r_max], f16, tag="SgA")
                SgB = sm.tile([P, 6 * r_max], f16, tag="SgB")
                for cums, smp in ((cB, smpB), (cA, smpA)):
                    cT = cums[:, :Ft].rearrange("p (r a) -> p a r", a=A)
                    smp3 = smp[:, :S6].rearrange("p (j r) -> p j r", j=6)
                    nc.vector.tensor_copy(smp3[:, 0:2], cT[:, 2:6:3])
                    nc.vector.tensor_copy(smp3[:, 2:5], cT[:, 9:60:25])
                    nc.vector.tensor_copy(smp3[:, 5:6], cT[:, 67:68])
                for smp, Sg in ((smpB, SgB), (smpA, SgA)):
                    # scan resets per row, so S_0 = smp0 directly.
                    nc.vector.tensor_copy(Sg[:, 0:rt], smp[:, 0:rt])
                    nc.vector.tensor_tensor(
                        Sg[:, rt:S6], smp[:, rt:S6], smp[:, :5 * rt], op=sub_op)
                # Z = SgB + mz*n_j (per-group const): fp32 out for recip/Ln.
                Zc = zp.tile([P, 6 * r_max], f32, tag="Zc")
                for g, (j0, k) in enumerate(GRP):
                    nc.vector.tensor_scalar(
                        Zc[:, j0 * rt:(j0 + k) * rt],
                        SgB[:, j0 * rt:(j0 + k) * rt],
                        float(MZ * GRP_N[g]), None, op0=add_op)
                rz = sm.tile([P, 6 * r_max], f32, tag="rz")
                nc.vector.reciprocal_approx_fast(rz[:, :S6], Zc[:, :S6])
                to = sm.tile([P, 6 * r_max], f32, tag="to")
                L = sm.tile([P, 6 * r_max], f16, tag="L")
                for g, (j0, k) in enumerate(GRP):
                    sl6 = slice(j0 * rt, (j0 + k) * rt)
                    # accA_g += sum (SgA + m*n_g) * (1/Z)
                    nc.vector.scalar_tensor_tensor(
                        out=to[:, sl6], in0=SgA[:, sl6], scalar=mn_ap[g],
                        in1=rz[:, sl6], op0=add_op, op1=mult_op,
                        accum_out=accA[:, t * 4 + g:t * 4 + g + 1])
                    nc.scalar.activation(
                        L[:, sl6], Zc[:, sl6], Ln,
                        accum_out=accB[:, t * 4 + g:t * 4 + g + 1])
            nc.sync.dma_start(acc_a[:], accA[:])
            nc.sync.dma_start(acc_b[:], accB[:])
    with _force_exp_ln_one_table_set():
        nc.compile()
    return nc, Tt


def _force_exp_ln_one_table_set():
    """Map Exp and Ln (and Copy, which the set already contains) to the single
    natural_log_exp_and_others table so ScalarE never reloads act tables."""
    import contextlib
    import concourse.bacc as bacc_mod
    import concourse.mybir as mybir

    @contextlib.contextmanager
    def ctx():
        orig = bacc_mod.get_activation_tables

        def patched(arch):
            tables = {k: set(v) for k, v in orig(arch).items()}
            for name, funcs in tables.items():
                if name != "natural_log_exp_and_others":
                    funcs.discard(mybir.ActivationFunctionType.Exp)
                    funcs.discard(mybir.ActivationFunctionType.Ln)
            return tables

        bacc_mod.get_activation_tables = patched
        try:
            yield
        finally:
            bacc_mod.get_activation_tables = orig

    return ctx()


def _get_program():
    key = (W_CORE, R)
    if key not in _PROGRAM_CACHE:
        _PROGRAM_CACHE[key] = build_program(W_CORE, R)
    return _PROGRAM_CACHE[key]


def _host_x(current_action):
    """Segmented log_softmax of current_action in float64 on host."""
    ca = np.asarray(current_action, np.float64)
    x = np.empty(A, np.float64)
    for o, n in zip(OFFS, NVEC):
        seg = ca[o:o + n]
        mx = seg.max()
        x[o:o + n] = seg - (mx + np.log(np.exp(seg - mx).sum()))
    return x


def _x_corr(x):
    """Expected bias from the fp16 quantization of x, removed host-side.

    The device computes d with x16 = fp16(x); the excess in the loss is
    sum_rows sum_j inv_n_j * sum_{i in j} w_i * (x_i - x16_i) with softmax
    weights w.  E[w_i] = 1/n_j for iid inputs, so the expected excess per
    row is sum_j (1/n_j^2) * sum_{i in j} delta_i (exact algebra otherwise
    untouched; residual is O(1e-5))."""
    delta = np.asarray(x, np.float64) - np.asarray(x, np.float32).astype(
        np.float16).astype(np.float64)
    return sum((1.0 / (n * n)) * delta[o:o + n].sum()
               for o, n in zip(OFFS, NVEC))


def combine_partials(results, w_full=W_FULL, x_corr=0.0):
    """accA = per-group sums of S/Z; accB = per-group sums of ln Z.
    loss = (1/W) * sum_g inv_n_g * (accA_g - accB_g) - x_corr."""
    inv_g = np.asarray([1.0 / 3, 1.0 / 4, 1.0 / 25, 1.0 / 8], np.float64)
    total = 0.0
    for res in results:
        a = np.asarray(res["acc_a"], np.float64).reshape(P, -1, 4).sum((0, 1))
        b = np.asarray(res["acc_b"], np.float64).reshape(P, -1, 4).sum((0, 1))
        total += (inv_g * (a - b)).sum()
    return np.float32(total / w_full - x_corr)


def _make_xbt(current_action):
    """xb payload: x (68) ++ m ++ 2m ++ m*n_g (4), broadcast to P rows."""
    x = _host_x(current_action)
    m = float(np.exp(0.5) * (1.0 - x.mean()))
    row = np.concatenate([
        x.astype(np.float32),
        np.asarray([m, 2 * m] + [m * n for n in GRP_N], np.float32)])
    return np.broadcast_to(row, (P, A + 6)).copy()


def kernel(current_action, previous_actions):
    from concourse import bass_utils

    nc, _ = _get_program()
    xbt = _make_xbt(current_action)
    pa = np.ascontiguousarray(np.asarray(previous_actions, np.float32))
    assert pa.shape == (W_FULL, A)
    in_maps = [
        {"pa": pa[c * W_CORE:(c + 1) * W_CORE], "xb": xbt}
        for c in range(N_CORES)
    ]
    res = bass_utils.run_bass_kernel_spmd(
        nc, in_maps, core_ids=list(range(N_CORES)))
    return combine_partials(
        res.results, x_corr=_x_corr(_host_x(current_action)))


if __name__ == "__main__":
    np.random.seed(0)
    ca = np.random.randn(A).astype(np.float32)
    pa = np.random.randn(W_FULL, A).astype(np.float32)
    print(kernel(ca, pa))


# revision 14
# speedup vs baseline: 1.3807x; 1.0290x over previous
"""Trainium2 Bass kernel for nn_ActionSmoothingLoss (v2: 2x DVE scans).

Math (per row y of previous_actions, x = segmented log_softmax(current_action)):
    e = exp(y);  d = y - x
    S_j = sum_{i in seg j} e_i d_i;  Z_j = sum_{i in seg j} e_i
    loss = (1/W) * sum_rows sum_j (1/n_j) * [ S_j / Z_j - log Z_j ]

v2 strategy (baseline v1 = 147.7us, DVE-bound with 3 full 1x passes):
  All three full-tile DVE passes run at 2 elem/cycle (2X_1PORT perf mode):
    - d = y16 - xrep   : builtin tensor_tensor, fp16 in/out (firmware has a
                         2x uop for TT) -- needs y in fp16, which ScalarE
                         produces with an extra Copy pass (ScalarE has slack).
    - cA = cumsum(e*d - m)  : custom DVE op MUL_CUMSUM_C2_ANT with a
                         HAND-AUTHORED 2x uop program (the repo's custom-DVE
                         framework ships 1x only -- "T1" in its design doc;
                         the table-gen side already supports uops_2x).
    - cB = cumsum(e - mz)   : custom CUMSUM_C2_ANT, also 2x. rd1 is force-
                         enabled (dummy in1=e) so the perf-mode byte is
                         TwoSrc and the un-authorable 2-port modes are
                         unreachable.
  2x requires 16-bit in AND out.  fp16 output of a raw cumsum would be
  numerically dead (c grows to ~1e4), so the scans are RECENTERED: the body
  subtracts a host-computed drift constant per element (m ~ E[e*d], mz =
  E[e] = sqrt(e)); the running sum stays O(100) and fp16 sampling error
  telescopes out of the final sum (numpy model: rel err ~3e-6).  The exact
  algebra S_j = diff_j + m*n_j holds for ANY m, so this is not an
  approximation -- m only controls rounding magnitude.  m depends on x, so
  it rides in xbt and reaches the op as a [P,1] AP scalar; mz is static.
  Z_j = diffB_j + mz*n_j is applied by 4 per-inv_n-group tensor_scalar adds
  (fp32 out, feeds reciprocal_approx_fast and ScalarE Ln+accum); the S
  correction folds into scalar_tensor_tensor's op0 (scalar = m*n_g AP).
  Sampling: cums viewed [P, 68, rt]; segment-end cols {2,5},{9,34,59},{67}
  -> smp [P, 6, rt] (j-major) in 3 strided copies; adjacent diff
  smp[1:6]-smp[0:5] is contiguous fp16 -> one 2x sub; the j=0 column uses
  the previous row's last sample (shifted by one) + first-element copy.
Partial sums accA/accB [P, T*4] (per inv_n group); host combines in f64
(inv_n applied host-side).
"""

import sys

sys.path.insert(0, "/opt/trn_rl_repo")

import numpy as np

NVEC = (3, 3, 4, 25, 25, 8)
OFFS = (0, 3, 6, 10, 35, 60)
ENDS = (2, 5, 9, 34, 59, 67)  # inclusive end column of each segment
# inv_n groups: segments {0,1} n=3, {2} n=4, {3,4} n=25, {5} n=8
GRP = ((0, 2), (2, 1), (3, 2), (5, 1))  # (first seg j, count) per group
GRP_N = (3, 4, 25, 8)
A = 68
P = 128
N_CORES = 8
W_FULL = 524288
W_CORE = W_FULL // N_CORES  # 65536
R = 64                      # base rows-per-partition unit for tile schedule
MZ = float(np.exp(0.5))     # E[exp(y)] for y~N(0,1): recenter const of cB

_PROGRAM_CACHE = {}
_OPS = None


def _fsm(seed, steady, step):
    """Wire the 3-state per-page machine: SEED -(count 1)-> STEADY; STEADY
    exits on SRC_TENSOR_DONE, jumps to STEP on SUB_DIM_DONE (page = one row
    of 68); STEP re-seeds the recurrence for one element(pair) then returns.
    Mirrors lower()'s segmented-scan machine / tensor_paged_mask firmware."""
    from concourse.dve_uop import Trigger
    seed.trigger = (Trigger.COUNT, Trigger.NONE, Trigger.NONE)
    seed.repeat_count = 1
    seed.next_uop = (1, 0, 0)
    steady.trigger = (Trigger.SRC_TENSOR_DONE, Trigger.SUB_DIM_DONE,
                      Trigger.NONE)
    steady.next_uop = (0, 2, 0)
    step.trigger = (Trigger.SRC_TENSOR_DONE, Trigger.SUB_DIM_DONE,
                    Trigger.COUNT)
    step.next_uop = (0, 2, 1)
    step.repeat_count = 1
    return [seed, steady, step]


def _build_mul_cumsum_1x():
    """1x 3-uop program for per-row-reset scan(ADD, Src0*Src1 - C0)."""
    import copy
    from concourse.dve_uop import (
        ENABLE, AluInp, AluOp, InpSel, OutPath, OutSel, UopConfig,
    )

    seed = UopConfig()
    seed.enable_input(InpSel.ZERO, 1)
    for b in range(2):
        seed.datapath_config[b].pass_through_delay(0)
    seed.datapath_config[2].enable_alu(AluOp.BYPASS, AluInp.PREV_DELAY_0)

    st = UopConfig()
    st.enable_input(InpSel.SRC_0, 1)               # ch0 = a
    st.enable_input(InpSel.SRC_1, 2)               # ch1 = b
    st.enable_input(InpSel.CONST_0, 3)             # ch2 = m
    blk = st.datapath_config
    blk[0].enable_alu(AluOp.MULTIPLY, AluInp.PREV_DELAY_0, AluInp.PREV_DELAY_1)
    blk[0].pass_through_delay(2)
    blk[1].enable_alu(AluOp.SUBTRACT, AluInp.PREV_ALU_OUT, AluInp.PREV_DELAY_2)
    blk[2].enable_alu(AluOp.ADD, AluInp.CURR_ALU_OUT, AluInp.PREV_ALU_OUT)
    for b in range(3, 8):
        blk[b].pass_through_alu()
    st.enable_output(OutSel.ALU_OUT, OutPath.WR0_LO)
    st.require_inp0 = st.require_inp1 = ENABLE

    step = copy.deepcopy(st)
    step.datapath_config[2].enable_alu(AluOp.BYPASS, AluInp.PREV_ALU_OUT)
    return _fsm(seed, st, step)


def _build_mul_cumsum_2x():
    """2X_1PORT 3-uop program for per-row-reset scan(ADD, Src0*Src1 - C0).

    Per cycle the engine delivers the packed pair (a0,b0),(a1,b1) as
    SRC_0/SRC_1/SRC_0_HI/SRC_1_HI.  Dataflow (one wavefront/cycle):
        p0 = a0*b0 ; p1 = a1*b1 ; s = p0+p1 ; s2 = s - C1   (C1 MUST be 2*C0)
        acc_hi = acc_hi' + s2        (1-cycle recurrence on block 4)
        acc_lo = acc_hi - p1 + C0
    WR0_LO <- acc_lo (elem 2i), WR0_HI <- acc_hi (elem 2i+1).  STEP resets
    the recurrence (acc_hi = s2) for the first pair of each 68-col row."""
    import copy
    from concourse.dve_uop import (
        ENABLE, AluInp, AluOp, DelayInp, InpSel, OutPath, OutSel, UopConfig,
    )

    seed = UopConfig()
    seed.enable_input(InpSel.ZERO, 1)              # chain0 = 0
    for b in range(4):
        seed.datapath_config[b].pass_through_delay(0)
    seed.datapath_config[4].enable_alu(AluOp.BYPASS, AluInp.PREV_DELAY_0)

    st = UopConfig()
    st.enable_input(InpSel.SRC_0, 1)               # ch0 = a0
    st.enable_input(InpSel.SRC_1, 2)               # ch1 = b0
    st.enable_input(InpSel.SRC_0_HI, 3)            # ch2 = a1
    st.enable_input(InpSel.SRC_1_HI, 4)            # ch3 = b1
    st.enable_input(InpSel.CONST_1, 5)             # ch4 = C1 = 2m
    st.enable_input(InpSel.CONST_0, 6)             # ch5 = C0 = m
    blk = st.datapath_config
    blk[0].enable_alu(AluOp.MULTIPLY, AluInp.PREV_DELAY_0, AluInp.PREV_DELAY_1)
    blk[0].pass_through_delay(2, 3, 4, 5)
    blk[1].enable_alu(AluOp.MULTIPLY, AluInp.PREV_DELAY_2, AluInp.PREV_DELAY_3)
    blk[1].enable_delay_from_src(DelayInp.PREV_ALU_OUT, 0)   # ch0 <- p0
    blk[1].pass_through_delay(4, 5)
    blk[2].enable_alu(AluOp.ADD, AluInp.PREV_ALU_OUT, AluInp.PREV_DELAY_0)
    blk[2].enable_delay_from_src(DelayInp.PREV_ALU_OUT, 1)   # ch1 <- p1
    blk[2].pass_through_delay(4, 5)
    blk[3].enable_alu(AluOp.SUBTRACT, AluInp.PREV_ALU_OUT, AluInp.PREV_DELAY_4)
    blk[3].pass_through_delay(1, 5)
    blk[4].enable_alu(AluOp.ADD, AluInp.CURR_ALU_OUT, AluInp.PREV_ALU_OUT)
    blk[4].pass_through_delay(1, 5)
    blk[5].enable_alu(AluOp.SUBTRACT, AluInp.PREV_ALU_OUT, AluInp.PREV_DELAY_1)
    blk[5].enable_delay_from_src(DelayInp.PREV_ALU_OUT, 0)   # ch0 <- acc_hi
    blk[5].pass_through_delay(5)
    blk[6].enable_alu(AluOp.ADD, AluInp.PREV_ALU_OUT, AluInp.PREV_DELAY_5)
    blk[6].pass_through_delay(0)
    blk[7].pass_through_alu()
    blk[7].pass_through_delay(0)
    st.enable_output(OutSel.ALU_OUT, OutPath.WR0_LO)
    st.enable_output(OutSel.DELAY_0, OutPath.WR0_HI)
    st.require_inp0 = st.require_inp1 = ENABLE

    step = copy.deepcopy(st)
    step.datapath_config[4].enable_alu(AluOp.BYPASS, AluInp.PREV_ALU_OUT)
    return _fsm(seed, st, step)


def _build_cumsum_1x():
    """1x 3-uop program for per-row-reset scan(ADD, Src0 - C0); CONSUMES a
    dummy src1 (rd1_en forced on so the perf-mode byte reads TwoSrc and the
    un-authored 2-port modes are unreachable)."""
    import copy
    from concourse.dve_uop import (
        ENABLE, AluInp, AluOp, InpSel, OutPath, OutSel, UopConfig,
    )

    seed = UopConfig()
    seed.enable_input(InpSel.ZERO, 1)
    seed.datapath_config[0].pass_through_delay(0)
    seed.datapath_config[1].enable_alu(AluOp.BYPASS, AluInp.PREV_DELAY_0)

    st = UopConfig()
    st.enable_input(InpSel.SRC_0, 1)               # ch0 = a
    st.enable_input(InpSel.CONST_0, 2)             # ch1 = m
    blk = st.datapath_config
    blk[0].enable_alu(AluOp.SUBTRACT, AluInp.PREV_DELAY_0, AluInp.PREV_DELAY_1)
    blk[1].enable_alu(AluOp.ADD, AluInp.CURR_ALU_OUT, AluInp.PREV_ALU_OUT)
    for b in range(2, 8):
        blk[b].pass_through_alu()
    st.enable_output(OutSel.ALU_OUT, OutPath.WR0_LO)
    st.require_inp0 = st.require_inp1 = ENABLE

    step = copy.deepcopy(st)
    step.datapath_config[1].enable_alu(AluOp.BYPASS, AluInp.PREV_ALU_OUT)
    return _fsm(seed, st, step)


def _build_cumsum_2x():
    """2X_1PORT 3-uop program for per-row-reset scan(ADD, Src0 - C0); src1
    consumed but unread.
        s = a0 + a1 ; s2 = s - C1 (=2m) ; acc_hi = acc_hi' + s2  (block 2)
        acc_lo = acc_hi - a1 + C0"""
    import copy
    from concourse.dve_uop import (
        ENABLE, AluInp, AluOp, DelayInp, InpSel, OutPath, OutSel, UopConfig,
    )

    seed = UopConfig()
    seed.enable_input(InpSel.ZERO, 1)
    for b in range(2):
        seed.datapath_config[b].pass_through_delay(0)
    seed.datapath_config[2].enable_alu(AluOp.BYPASS, AluInp.PREV_DELAY_0)

    st = UopConfig()
    st.enable_input(InpSel.SRC_0, 1)               # ch0 = a0
    st.enable_input(InpSel.SRC_0_HI, 2)            # ch1 = a1
    st.enable_input(InpSel.CONST_1, 3)             # ch2 = 2m
    st.enable_input(InpSel.CONST_0, 4)             # ch3 = m
    blk = st.datapath_config
    blk[0].enable_alu(AluOp.ADD, AluInp.PREV_DELAY_0, AluInp.PREV_DELAY_1)
    blk[0].pass_through_delay(1, 2, 3)
    blk[1].enable_alu(AluOp.SUBTRACT, AluInp.PREV_ALU_OUT, AluInp.PREV_DELAY_2)
    blk[1].pass_through_delay(1, 3)
    blk[2].enable_alu(AluOp.ADD, AluInp.CURR_ALU_OUT, AluInp.PREV_ALU_OUT)
    blk[2].pass_through_delay(1, 3)
    blk[3].enable_alu(AluOp.SUBTRACT, AluInp.PREV_ALU_OUT, AluInp.PREV_DELAY_1)
    blk[3].enable_delay_from_src(DelayInp.PREV_ALU_OUT, 0)   # ch0 <- acc_hi
    blk[3].pass_through_delay(3)
    blk[4].enable_alu(AluOp.ADD, AluInp.PREV_ALU_OUT, AluInp.PREV_DELAY_3)
    blk[4].pass_through_delay(0)
    for b in range(5, 8):
        blk[b].pass_through_alu()
        blk[b].pass_through_delay(0)
    st.enable_output(OutSel.ALU_OUT, OutPath.WR0_LO)
    st.enable_output(OutSel.DELAY_0, OutPath.WR0_HI)
    st.require_inp0 = st.require_inp1 = ENABLE

    step = copy.deepcopy(st)
    step.datapath_config[2].enable_alu(AluOp.BYPASS, AluInp.PREV_ALU_OUT)
    return _fsm(seed, st, step)


def _register_ops():
    """Register MUL_CUMSUM_C2_ANT / CUMSUM_C2_ANT with 1x (lowered or hand)
    and hand-authored 2x programs; pre-seed the compile cache so table-gen
    ships the 2x entries.  CALLER INVARIANT: s1 must equal 2*s0 (the 2x
    program uses C1 for the pair-sum recenter).  Idempotent."""
    global _OPS
    if _OPS is not None:
        return _OPS
    import concourse.dve_ops as dve_ops_mod
    from concourse.dve_ops import _COMPILE_CACHE
    from concourse.dve_spec import AluOp, C0, Spec, Src0, Src1, scan
    from concourse.dve_uop import DveOpSpec

    def _c0(c0, nd):
        if np.isscalar(c0):
            return np.float32(c0)
        a = np.asarray(c0, np.float32)
        return a.reshape(a.shape[0], *([1] * (nd - 1)))

    def _ref_mc(in0, in1, c0, c1, imm2):
        # in0/out [P, S, N] (paged); in1 flat [P, S*N]; cumsum resets per page
        a0 = np.asarray(in0, np.float32)
        a1 = np.asarray(in1, np.float32).reshape(a0.shape)
        prod = a0 * a1 - _c0(c0, a0.ndim)
        return np.cumsum(prod, axis=-1, dtype=np.float32)

    def _ref_c(in0, in1, c0, c1, imm2):
        a0 = np.asarray(in0, np.float32)
        t = a0 - _c0(c0, a0.ndim)
        return np.cumsum(t, axis=-1, dtype=np.float32)

    out = []
    for name, body_mul, ref in (
        ("MUL_CUMSUM_C2_ANT", True, _ref_mc),
        ("CUMSUM_C2_ANT", False, _ref_c),
    ):
        existing = [op for op in dve_ops_mod.OPS if op.name == name]
        if existing:
            out.append(existing[0])
            continue
        # spec.body documents the elementwise semantics and feeds nothing but
        # the CoreSim reference (the per-row reset lives in the hand uops +
        # reference; lower() is not used).
        if body_mul:
            spec = Spec(body=scan(AluOp.ADD, Src0 * Src1 - C0), reference=ref)
            uops_1x = _build_mul_cumsum_1x()
            uops_2x = _build_mul_cumsum_2x()
        else:
            spec = Spec(body=scan(AluOp.ADD, Src0 - C0), reference=ref)
            uops_1x = _build_cumsum_1x()
            uops_2x = _build_cumsum_2x()
        row = dve_ops_mod._CUSTOM_DVE_ROW_BASE + len(dve_ops_mod.OPS)
        assert row < 0x20
        compiled = DveOpSpec(
            name=name, opcode=row, uops=uops_1x, uops_2x=uops_2x,
            rd1_en=True, perf_max=1,
        )
        for u in uops_1x + uops_2x:
            u.validate("v3")
        shas = {"v3": compiled.sha("v3")}
        op = dve_ops_mod.DveOp(name, spec, subdim=True, uops_sha=shas)
        dve_ops_mod.OPS.append(op)
        dve_ops_mod._SUB_OPCODE_FOR_NAME[name] = row
        dve_ops_mod.CUSTOM_DVE_SPECS[name] = spec
        _COMPILE_CACHE[(name, "v3")] = compiled
        out.append(op)
    _OPS = tuple(out)
    return _OPS


def build_program(w_core=W_CORE, r=R):
    import concourse.bass as bass
    import concourse.bacc as bacc
    import concourse.mybir as mybir
    from concourse import tile

    op_mc, op_c = _register_ops()

    f32 = mybir.dt.float32
    f16 = mybir.dt.float16
    rows_pp = w_core // P
    if rows_pp >= 384 and (rows_pp - 128) % 128 == 0:
        RS = [64, 64] + [128] * ((rows_pp - 128) // 128)
    else:
        RS = [64] * (rows_pp // 64)
    assert sum(RS) == rows_pp
    r_max = max(RS)
    Fmax = r_max * A
    XR = 64 * A  # xrep covers 64 rows; bigger tiles subtract in 64-row chunks
    Tt = len(RS)

    Exp = mybir.ActivationFunctionType.Exp
    Ln = mybir.ActivationFunctionType.Ln
    Copy = mybir.ActivationFunctionType.Copy
    sub_op = mybir.AluOpType.subtract
    add_op = mybir.AluOpType.add
    mult_op = mybir.AluOpType.mult

    nc = bacc.Bacc(None, target_bir_lowering=False)
    pa = nc.dram_tensor("pa", [w_core, A], f32, kind="ExternalInput")
    # xb: cols 0..67 = x (fp32); 68 = m; 69 = 2m; 70..73 = m*n_g per group.
    xb = nc.dram_tensor("xb", [P, A + 6], f32, kind="ExternalInput")
    # x16 replicated across 64 rows, host-prepared (saves an on-device cast).
    xr = nc.dram_tensor("xr", [P, XR], f16, kind="ExternalInput")
    acc_a = nc.dram_tensor("acc_a", [P, Tt * 4], f32, kind="ExternalOutput")
    acc_b = nc.dram_tensor("acc_b", [P, Tt * 4], f32, kind="ExternalOutput")

    pav = pa.rearrange("(p q) a -> p (q a)", p=P)

    def cdve(op, out, in0, in1, s0, s1):
        # perf_max must be set at construction (add_instruction copies the
        # instruction into the Rust module; post-hoc mutation is lost), so
        # wrap the class with a kwarg-injecting factory for this emit.
        from concourse import bass_isa as bi
        real = bi.InstCustomDveAnt

        def patched(*a, **kw):
            kw.setdefault("perf_max", 1)
            return real(*a, **kw)

        bi.InstCustomDveAnt = patched
        try:
            return nc.vector._custom_dve(
                op, out=out, in0=in0, in1=in1, s0=s0, s1=s1)
        finally:
            bi.InstCustomDveAnt = real

    with tile.TileContext(nc) as tc:
        with tc.tile_pool(name="ps", bufs=1) as ps, \
             tc.tile_pool(name="io", bufs=2) as io, \
             tc.tile_pool(name="ep", bufs=2) as ep, \
             tc.tile_pool(name="dp", bufs=1) as dp, \
             tc.tile_pool(name="cm", bufs=1) as cm, \
             tc.tile_pool(name="sm", bufs=1) as sm, \
             tc.tile_pool(name="zp", bufs=2) as zp:
            xbt = ps.tile([P, A + 6], f32)
            nc.sync.dma_start(xbt[:], xb[:], single_packet=True)
            m_ap = xbt[:, A:A + 1]
            m2_ap = xbt[:, A + 1:A + 2]
            mn_ap = [xbt[:, A + 2 + g:A + 3 + g] for g in range(4)]
            xrep = ps.tile([P, XR], f16)
            nc.sync.dma_start(xrep[:], xr[:])
            accA = ps.tile([P, Tt * 4], f32)
            accB = ps.tile([P, Tt * 4], f32)
            row0 = 0
            for t, rt in enumerate(RS):
                Ft = rt * A
                S6 = 6 * rt
                H = Ft // 2
                src = pav[:, row0 * A:(row0 + rt) * A]
                row0 += rt
                y = io.tile([P, Fmax], f32, tag="y")
                e = ep.tile([P, Fmax], f16, tag="e")
                d = dp.tile([P, Fmax], f16, tag="d")
                cA = cm.tile([P, Fmax], f16, tag="cA")
                cB = cm.tile([P, Fmax], f16, tag="cB")
                if t == 0:
                    Q = Ft // 4
                    sl = tuple((q * Q, (q + 1) * Q) for q in range(4))
                    for h0, h1 in sl:
                        nc.sync.dma_start(y[:, h0:h1], src[:, h0:h1])
                else:
                    sl = ((0, H), (H, Ft))
                    nc.sync.dma_start(y[:, :H], src[:, :H])
                    nc.sync.dma_start(y[:, H:Ft], src[:, H:])
                # ScalarE: e = exp(y) fp16; y16 = Copy(y) fp16 (into d; the
                # DVE subtract then runs in-place at 2x).
                for h0, h1 in sl:
                    nc.scalar.activation(e[:, h0:h1], y[:, h0:h1], Exp)
                for h0, h1 in sl:
                    nc.scalar.activation(d[:, h0:h1], y[:, h0:h1], Copy)
                # DVE big passes (all 2X_1PORT). Scans are per-row-reset:
                # in0/out are [P, rt, 68] paged APs (subdim ops); in1 rides
                # flat (TTSS struct, so C1 can be a [P,1] AP).
                e3 = e[:, :Ft].rearrange("p (r a) -> p r a", a=A)
                cdve(op_c, cB[:, :Ft].rearrange("p (r a) -> p r a", a=A),
                     e3, e[:, :Ft], MZ, 2.0 * MZ)
                for c0 in range(0, Ft, XR):
                    c1 = min(c0 + XR, Ft)
                    nc.vector.tensor_tensor(
                        d[:, c0:c1], d[:, c0:c1], xrep[:, :c1 - c0], op=sub_op)
                cdve(op_mc, cA[:, :Ft].rearrange("p (r a) -> p r a", a=A),
                     e3, d[:, :Ft], m_ap, m2_ap)
                # Per-(row,seg) sums straight from the cumsums (scan resets
                # per row): S_j = c[.,end_j] - c[.,end_{j-1}], S_0 = c[.,2].
                # Z side fuses the +mz*n_j correction into the diff via
                # scalar_tensor_tensor: Zc_j = (cB_j + mz*n_j) - cB_{j-1}.
                cA3 = cA[:, :Ft].rearrange("p (r a) -> p r a", a=A)
                cB3 = cB[:, :Ft].rearrange("p (r a) -> p r a", a=A)
                SgA = sm.tile([P, 6 * r_max], f16, tag="SgA")
                Zc = zp.tile([P, 6 * r_max], f32, tag="Zc")
                nc.vector.tensor_copy(
                    SgA[:, 0:rt].rearrange("p (r o) -> p r o", o=1),
                    cA3[:, :, 2:3])
                nc.vector.tensor_scalar(
                    Zc[:, 0:rt].rearrange("p (r o) -> p r o", o=1),
                    cB3[:, :, 2:3], float(MZ * 3), None, op0=add_op)
                for j in range(1, 6):
                    e1, e0 = ENDS[j], ENDS[j - 1]
                    nc.vector.tensor_tensor(
                        SgA[:, j * rt:(j + 1) * rt].rearrange(
                            "p (r o) -> p r o", o=1),
                        cA3[:, :, e1:e1 + 1], cA3[:, :, e0:e0 + 1], op=sub_op)
                    nc.vector.scalar_tensor_tensor(
                        out=Zc[:, j * rt:(j + 1) * rt].rearrange(
                            "p (r o) -> p r o", o=1),
                        in0=cB3[:, :, e1:e1 + 1],
                        scalar=float(MZ * NVEC[j]),
                        in1=cB3[:, :, e0:e0 + 1], op0=add_op, op1=sub_op)
                rz = sm.tile([P, 6 * r_max], f32, tag="rz")
                nc.vector.reciprocal_approx_fast(rz[:, :S6], Zc[:, :S6])
                to = sm.tile([P, 6 * r_max], f32, tag="to")
                L = sm.tile([P, 6 * r_max], f16, tag="L")
                for g, (j0, k) in enumerate(GRP):
                    sl6 = slice(j0 * rt, (j0 + k) * rt)
                    # accA_g += sum (SgA + m*n_g) * (1/Z)
                    nc.vector.scalar_tensor_tensor(
                        out=to[:, sl6], in0=SgA[:, sl6], scalar=mn_ap[g],
                        in1=rz[:, sl6], op0=add_op, op1=mult_op,
                        accum_out=accA[:, t * 4 + g:t * 4 + g + 1])
                    nc.scalar.activation(
                        L[:, sl6], Zc[:, sl6], Ln,
                        accum_out=accB[:, t * 4 + g:t * 4 + g + 1])
            nc.sync.dma_start(acc_a[:], accA[:])
            nc.sync.dma_start(acc_b[:], accB[:])
    with _force_exp_ln_one_table_set():
        nc.compile()
    return nc, Tt


def _force_exp_ln_one_table_set():
    """Map Exp and Ln (and Copy, which the set already contains) to the single
    natural_log_exp_and_others table so ScalarE never reloads act tables."""
    import contextlib
    import concourse.bacc as bacc_mod
    import concourse.mybir as mybir

    @contextlib.contextmanager
    def ctx():
        orig = bacc_mod.get_activation_tables

        def patched(arch):
            tables = {k: set(v) for k, v in orig(arch).items()}
            for name, funcs in tables.items():
                if name != "natural_log_exp_and_others":
                    funcs.discard(mybir.ActivationFunctionType.Exp)
                    funcs.discard(mybir.ActivationFunctionType.Ln)
            return tables

        bacc_mod.get_activation_tables = patched
        try:
            yield
        finally:
            bacc_mod.get_activation_tables = orig

    return ctx()


def _get_program():
    key = (W_CORE, R)
    if key not in _PROGRAM_CACHE:
        _PROGRAM_CACHE[key] = build_program(W_CORE, R)
    return _PROGRAM_CACHE[key]


def _host_x(current_action):
    """Segmented log_softmax of current_action in float64 on host."""
    ca = np.asarray(current_action, np.float64)
    x = np.empty(A, np.float64)
    for o, n in zip(OFFS, NVEC):
        seg = ca[o:o + n]
        mx = seg.max()
        x[o:o + n] = seg - (mx + np.log(np.exp(seg - mx).sum()))
    return x


def _x_corr(x):
    """Expected bias from the fp16 quantization of x, removed host-side.

    The device computes d with x16 = fp16(x); the excess in the loss is
    sum_rows sum_j inv_n_j * sum_{i in j} w_i * (x_i - x16_i) with softmax
    weights w.  E[w_i] = 1/n_j for iid inputs, so the expected excess per
    row is sum_j (1/n_j^2) * sum_{i in j} delta_i (exact algebra otherwise
    untouched; residual is O(1e-5))."""
    delta = np.asarray(x, np.float64) - np.asarray(x, np.float32).astype(
        np.float16).astype(np.float64)
    return sum((1.0 / (n * n)) * delta[o:o + n].sum()
               for o, n in zip(OFFS, NVEC))


def combine_partials(results, w_full=W_FULL, x_corr=0.0):
    """accA = per-group sums of S/Z; accB = per-group sums of ln Z.
    loss = (1/W) * sum_g inv_n_g * (accA_g - accB_g) - x_corr."""
    inv_g = np.asarray([1.0 / 3, 1.0 / 4, 1.0 / 25, 1.0 / 8], np.float64)
    total = 0.0
    for res in results:
        a = np.asarray(res["acc_a"], np.float64).reshape(P, -1, 4).sum((0, 1))
        b = np.asarray(res["acc_b"], np.float64).reshape(P, -1, 4).sum((0, 1))
        total += (inv_g * (a - b)).sum()
    return np.float32(total / w_full - x_corr)


def _make_xbt(current_action):
    """xb payload: x (68) ++ m ++ 2m ++ m*n_g (4), broadcast to P rows."""
    x = _host_x(current_action)
    m = float(np.exp(0.5) * (1.0 - x.mean()))
    row = np.concatenate([
        x.astype(np.float32),
        np.asarray([m, 2 * m] + [m * n for n in GRP_N], np.float32)])
    return np.broadcast_to(row, (P, A + 6)).copy()


def _make_xr(current_action):
    """xr payload: fp16 x tiled across 64 rows, broadcast to P partitions."""
    x16 = _host_x(current_action).astype(np.float32).astype(np.float16)
    return np.broadcast_to(np.tile(x16, 64), (P, 64 * A)).copy()


def kernel(current_action, previous_actions):
    from concourse import bass_utils

    nc, _ = _get_program()
    xbt = _make_xbt(current_action)
    xr = _make_xr(current_action)
    pa = np.ascontiguousarray(np.asarray(previous_actions, np.float32))
    assert pa.shape == (W_FULL, A)
    in_maps = [
        {"pa": pa[c * W_CORE:(c + 1) * W_CORE], "xb": xbt, "xr": xr}
        for c in range(N_CORES)
    ]
    res = bass_utils.run_bass_kernel_spmd(
        nc, in_maps, core_ids=list(range(N_CORES)))
    return combine_partials(
        res.results, x_corr=_x_corr(_host_x(current_action)))


if __name__ == "__main__":
    np.random.seed(0)
    ca = np.random.randn(A).astype(np.float32)
    pa = np.random.randn(W_FULL, A).astype(np.float32)
    print(kernel(ca, pa))
